# revision 31
# baseline (speedup 1.0000x reference)
# Multi-head attention (B=2, S=2048, D=1024, H=16) on 8 TRN2 NeuronCores.
#
# Sharding: core c handles batch b = c//4 and head-group hg = c%4 (4 heads,
# channel slice J = hg*256 : (hg+1)*256).  Each core computes
#   Q^T/K^T = W^T x^T (+bias), V = x W (+bias),
#   S^T_h = K_h^T^T-contraction (d on partitions)  -> exp on ScalarE,
#   O^T_h = [V | 1]^T P^T_h  (row 64 = softmax denominator),
#   y_partial = O^T^T Wo_slice    (bf16, [S, D])
# Host sums the 4 partials per batch and adds bo.
#
# Schedule (v2): the PE stream is the binding path, so everything is
# organized to keep it gapless from ~10us on:
#  - xT and Wqkv live in single SBUF tiles [128, KT, .] so one DMA
#    instruction covers a column-slice of ALL k-tiles (3D DRAM AP).  The
#    pair-0 wq/wk columns load first (0.5MB prefix), then xt streams in
#    seq-quarters; the first scores unit starts at ~14us instead of ~32us.
#  - Weight/bias/wv DMAs ride the Scalar engine's HWDGE queue so the sync
#    engine's descriptor time stays off the critical path.
#  - Unit order (0,0),(0,1),(0,2),(1,0),(1,1),(1,2),(0,3),(1,3): y(c)
#    unlocks after the second pair of chunk c, spreading output DMAs.
#  - Pump generators are drained into the exp slots in deadline order
#    (emission order == execution order per engine); unit 1 pumps hard
#    (PE-paced) to finish V + the remaining kt0 chunks before attn@V of
#    unit 1 consumes them during unit 2.
#  - Tail: the last unit interleaves its own attn@V one slot behind the
#    exp stream; a reserved half of y(c2) covers the final normalize
#    latency so the PE p-state never drops before the last y matmuls.
#
# All matmuls bf16; scores use K=64 tile packing (two heads' MMs run
# CONCURRENT in disjoint PE row groups).  Q/K biases fold into the
# PSUM->SBUF cast (per-partition tensor_scalar add); V bias folds into its
# cast.  Softmax: denominator row 64 bounces through DRAM to replicate
# across partitions, then reciprocal_approx_fast.

import numpy as np

B = 2
S = 2048
D = 1024
H = 16
DH = 64
NCORES = 8
HL = 4            # heads per core
J = HL * DH       # 256: per-core channel slice of D
PAIRS = 2         # head-pairs per core

_cache = {}


def _build_module(seq=S):
    import concourse.bass as bass
    import concourse.mybir as mybir
    import concourse.tile as tile

    from concourse import bacc

    dt = mybir.dt
    f32 = dt.float32
    bf16 = dt.bfloat16
    AF = mybir.ActivationFunctionType

    KB = seq // 128          # key blocks (partition tiles of the key dim)
    QC = min(512, seq)       # query chunk (matmul free dim)
    NQ = seq // QC           # query chunks
    NCH = min(512, seq)      # projection free-dim chunk
    NP = seq // NCH          # projection chunks
    KT = D // 128            # contraction tiles for projections (8)

    nc = bacc.Bacc("TRN2", target_bir_lowering=False, debug=False)

    xT_d = nc.dram_tensor("xT", [D, seq], bf16, kind="ExternalInput").ap()
    # Weight slabs pre-transposed on host to [128, KT*256] (partition-major)
    # so each loads with ONE contiguous 2D DMA: w0/w1 = [wq_p|wk_p] of head
    # pair p, wv = the V weights.
    w0_d = nc.dram_tensor("w0", [128, KT * 256], bf16, kind="ExternalInput").ap()
    w1_d = nc.dram_tensor("w1", [128, KT * 256], bf16, kind="ExternalInput").ap()
    wv_d = nc.dram_tensor("wv", [128, KT * 256], bf16, kind="ExternalInput").ap()
    wo_d = nc.dram_tensor("wo", [J, D], bf16, kind="ExternalInput").ap()
    bqk_d = nc.dram_tensor("bqk", [128, 4], f32, kind="ExternalInput").ap()
    # bv pre-replicated to all 128 partitions on host.
    bv_d = nc.dram_tensor("bv", [128, J], bf16, kind="ExternalInput").ap()
    y_d = nc.dram_tensor("y", [seq, D], bf16, kind="ExternalOutput").ap()

    with tile.TileContext(nc) as tc:
        import contextlib
        ctx = contextlib.ExitStack()
        with ctx:
            xt_pool = ctx.enter_context(tc.tile_pool(name="xt", bufs=1))
            w_pool = ctx.enter_context(tc.tile_pool(name="w", bufs=1))
            qk_pool = ctx.enter_context(tc.tile_pool(name="qk", bufs=1))
            v_pool = ctx.enter_context(tc.tile_pool(name="v", bufs=1))
            pt_pool = ctx.enter_context(tc.tile_pool(name="pt", bufs=2))
            ot_pool = ctx.enter_context(tc.tile_pool(name="ot", bufs=1))
            sm_pool = ctx.enter_context(tc.tile_pool(name="sm", bufs=5))
            yb_pool = ctx.enter_context(tc.tile_pool(name="yb", bufs=4))
            psS_pool = ctx.enter_context(
                tc.tile_pool(name="psS", bufs=2, space="PSUM"))
            psO_pool = ctx.enter_context(
                tc.tile_pool(name="psO", bufs=2, space="PSUM"))
            mm_pool = ctx.enter_context(
                tc.tile_pool(name="mm", bufs=2, space="PSUM"))
            dram_pool = ctx.enter_context(
                tc.tile_pool(name="dscr", bufs=4, space="DRAM"))

            # ---- persistent SBUF tensors ----
            xt_all = xt_pool.tile([128, KT, seq], bf16, tag="xt", name="xt")
            xt_sb = [xt_all[:, k, :] for k in range(KT)]
            # wp_sb[p][:, k, 0:128] = wq pair p, [:, k, 128:256] = wk pair p
            wp_sb = [w_pool.tile([128, KT, 256], bf16, tag=f"w{p}",
                                 name=f"w{p}") for p in range(PAIRS)]
            wv_all = w_pool.tile([128, KT, 256], bf16, tag="wv", name="wv")
            wv_sb = [wv_all[:, k, :] for k in range(KT)]
            wo_sb = [w_pool.tile([128, D], bf16, tag=f"wo{p}",
                                 name=f"wo{p}") for p in range(PAIRS)]
            bqk_sb = w_pool.tile([128, 4], f32, tag="bqk", name="bqk")
            bvr_sb = w_pool.tile([128, J], bf16, tag="bvr", name="bvr")
            dz_sb = w_pool.tile([128, 512], bf16, tag="dz", name="dz")

            qt_sb = [qk_pool.tile([128, seq], bf16, tag=f"qt{p}",
                                  name=f"qt{p}") for p in range(PAIRS)]
            kt_sb = [qk_pool.tile([128, seq], bf16, tag=f"kt{p}",
                                  name=f"kt{p}") for p in range(PAIRS)]
            # V padded to 128 columns per head (NumWeights==128 -> FWL).
            v_sb = [v_pool.tile([128, HL, 128], bf16, tag=f"v{s}",
                                name=f"v{s}") for s in range(KB)]
            ot_sb = [ot_pool.tile([128, seq], bf16, tag=f"ot{p}",
                                  name=f"ot{p}") for p in range(PAIRS)]

            # ---- input DMAs ----
            def xtq(q, k0, k1):
                """One DMA for xt quarter q, k-tiles k0..k1-1."""
                x0 = xT_d[0:1, 0:1]
                nc.sync.dma_start(
                    out=xt_all[:, k0:k1, q * QC:(q + 1) * QC],
                    in_=bass.AP(tensor=x0.tensor,
                                offset=x0.offset + k0 * 128 * seq + q * QC,
                                ap=[[seq, 128], [128 * seq, k1 - k0],
                                    [1, QC]]))

            # Sync queue, deadline order: pair-0 weights + xt quarter 0
            # (gate the prefix chains), bqk (prefix bias), wv + bv (V
            # blocks from ~slot 2), then the later quarters and weights.
            # Sync HWDGE queue (one queue sustains only ~100-150 GB/s
            # serially): the prefix w0 + xt stream in deadline order.  The
            # late-needed weights (w1, wo: ~1MB) ride the gpsimd SWDGE
            # queue in parallel so they don't delay the xt quarters.
            nc.sync.dma_start(out=wp_sb[0], in_=w0_d)
            for kp in range(KT // 2):
                xtq(0, 2 * kp, 2 * kp + 2)
            nc.sync.dma_start(out=bqk_sb, in_=bqk_d)
            nc.sync.dma_start(out=wv_all, in_=wv_d)
            nc.sync.dma_start(out=bvr_sb, in_=bv_d)
            for kp in range(KT // 2):
                xtq(1, 2 * kp, 2 * kp + 2)
            nc.sync.dma_start(out=wp_sb[1], in_=w1_d)
            xtq(2, 0, KT // 2)
            xtq(2, KT // 2, KT)
            for pp in range(PAIRS):
                nc.sync.dma_start(out=wo_sb[pp],
                                  in_=wo_d[pp * 128:(pp + 1) * 128, :])
            xtq(3, 0, KT // 2)
            xtq(3, KT // 2, KT)

            # PE p-state warm-up: dummy matmuls on a zeroed tile ramp the
            # tensor clock while the first DMAs land.
            nc.vector.memset(dz_sb, 0.0)
            trash = psS_pool.tile([128, 2, QC], f32, tag="psS", name="trash")
            for i in range(12):
                nc.tensor.matmul(trash[:, 0, :], lhsT=dz_sb[:, 0:128],
                                 rhs=dz_sb, start=True, stop=True)
            for s in range(KB):
                nc.vector.memset(v_sb[s][:, :, DH:DH + 1], 1.0)
                nc.vector.memset(v_sb[s][:, :, DH + 1:], 0.0)
            # Warm the exp table set during the DMA ramp.
            warm = w_pool.tile([1, 8], f32, tag="warm", name="warm")
            nc.vector.memset(warm, 0.0)
            nc.scalar.activation(out=warm, in_=warm, func=AF.Exp)

            def wq_s(p, k):
                return wp_sb[p][:, k, 0:128]

            def wk_s(p, k):
                return wp_sb[p][:, k, 128:256]

            # ---- prefix: kt0 chunk 0 + qt0 chunk 0, k-outer (DMA-paced) ----
            ps_k0 = mm_pool.tile([128, 512], f32, tag="mm", name="pfx_k")
            ps_q0 = mm_pool.tile([128, 512], f32, tag="mm", name="pfx_q")
            for k in range(KT):
                nc.tensor.matmul(ps_k0[:, :NCH], lhsT=wk_s(0, k),
                                 rhs=xt_sb[k][:, 0:NCH],
                                 start=(k == 0), stop=(k == KT - 1))
                nc.tensor.matmul(ps_q0[:, :NCH], lhsT=wq_s(0, k),
                                 rhs=xt_sb[k][:, 0:NCH],
                                 start=(k == 0), stop=(k == KT - 1))
            nc.vector.tensor_scalar_add(kt_sb[0][:, 0:NCH], ps_k0[:, :NCH],
                                        bqk_sb[:, 2:3])
            nc.vector.tensor_scalar_add(qt_sb[0][:, 0:NCH], ps_q0[:, :NCH],
                                        bqk_sb[:, 0:1])

            # ---- emission helpers ----
            def gen_qk_chunk(which, p, nck):
                """One Q^T (which=0) / K^T (which=1) chunk, k-inner."""
                w_f = wq_s if which == 0 else wk_s
                dst = qt_sb[p] if which == 0 else kt_sb[p]
                bcol = which * 2 + p
                ps = mm_pool.tile([128, 512], f32, tag="mm",
                                  name=f"psqk{which}{p}{nck}")
                for k in range(KT):
                    nc.tensor.matmul(
                        ps[:, :NCH],
                        lhsT=w_f(p, k),
                        rhs=xt_sb[k][:, nck * NCH:(nck + 1) * NCH],
                        start=(k == 0), stop=(k == KT - 1))
                    yield
                nc.vector.tensor_scalar_add(
                    dst[:, nck * NCH:(nck + 1) * NCH], ps[:, :NCH],
                    bqk_sb[:, bcol:bcol + 1])
                yield

            def gen_v(s0, s1):
                """V blocks s0..s1-1; bias added during the psum cast."""
                for s in range(s0, s1):
                    ps = mm_pool.tile([128, 512], f32, tag="mm", name=f"psv{s}")
                    for k in range(KT):
                        nc.tensor.matmul(
                            ps[:, :J],
                            lhsT=xt_sb[k][:, s * 128:(s + 1) * 128],
                            rhs=wv_sb[k],
                            start=(k == 0), stop=(k == KT - 1))
                        yield
                    nc.vector.tensor_add(
                        v_sb[s][:, :, 0:DH],
                        ps[:, :J].rearrange("p (h d) -> p h d", h=HL),
                        bvr_sb.rearrange("p (h d) -> p h d", h=HL))
                    yield

            pt_tiles = {}

            class Gen:
                def __init__(self, it):
                    self.it = it
                    self.done = False

                def step(self):
                    if self.done:
                        return False
                    try:
                        next(self.it)
                        return True
                    except StopIteration:
                        self.done = True
                        return False

            pending = []

            def pump(n):
                while n > 0 and pending:
                    if pending[0].step():
                        n -= 1
                    else:
                        pending.pop(0)

            def drain(g):
                while g.step():
                    pass

            def emit_sT(p, c, av, base_pump=4, av_pump=2):
                """Scores^T + exp for head-pair p, query chunk c."""
                pt = pt_pool.tile([128, KB, 2, QC], bf16, tag="pt",
                                  name=f"pt{p}{c}")
                pt_tiles[(p, c)] = pt
                for kb in range(KB):
                    ps = psS_pool.tile([128, 2, QC], f32, tag="psS",
                                       name=f"psS{p}{c}{kb}")
                    for h01 in range(2):
                        nc.tensor.matmul(
                            ps[:, h01, :],
                            lhsT=kt_sb[p][h01 * 64:(h01 + 1) * 64,
                                          kb * 128:(kb + 1) * 128],
                            rhs=qt_sb[p][h01 * 64:(h01 + 1) * 64,
                                         c * QC:(c + 1) * QC],
                            start=True, stop=True,
                            tile_position=(h01 * 64, 0))
                    nc.scalar.activation(
                        out=pt[:, kb, :, :], in_=ps,
                        func=AF.Exp, scale=0.125)
                    if av is not None and not av.done:
                        av.step()
                        av.step()
                        pump(av_pump)
                    else:
                        pump(base_pump)

            def gen_av(p, c):
                """attn @ [V|1], reciprocal, normalize, build O^T pair tile."""
                pt = pt_tiles.pop((p, c))
                for h01 in range(2):
                    h = p * 2 + h01
                    pso = psO_pool.tile([128, QC], f32, tag="psO",
                                        name=f"psO{p}{c}{h01}")
                    for kb in range(KB):
                        nc.tensor.matmul(
                            pso,
                            lhsT=v_sb[kb][:, h, :],
                            rhs=pt[:, kb, h01, :],
                            start=(kb == 0), stop=(kb == KB - 1))
                        yield
                    osb = sm_pool.tile([DH + 1, QC], f32, tag="osb",
                                       name=f"osb{p}{c}{h01}")
                    nc.vector.tensor_copy(osb, pso[0:DH + 1, :])
                    yield
                    ds = dram_pool.tile([1, QC], f32, tag="ds",
                                        name=f"ds{p}{c}{h01}")
                    nc.sync.dma_start(out=ds, in_=osb[DH:DH + 1, :])
                    dsap = ds[0:1, :]
                    rbs = sm_pool.tile([64, QC], f32, tag="rbs",
                                       name=f"rbs{p}{c}{h01}")
                    nc.sync.dma_start(
                        out=rbs,
                        in_=bass.AP(tensor=dsap.tensor, offset=dsap.offset,
                                    ap=[[0, 64], [1, QC]]))
                    rb = sm_pool.tile([64, QC], f32, tag="rb",
                                      name=f"rb{p}{c}{h01}")
                    nc.vector.reciprocal_approx_fast(out=rb, in_=rbs)
                    if h01 == 0:
                        nc.vector.tensor_mul(
                            ot_sb[p][0:64, c * QC:(c + 1) * QC],
                            osb[0:DH, :], rb)
                    else:
                        tmp = sm_pool.tile([64, QC], bf16, tag="ottmp",
                                           name=f"ottmp{p}{c}")
                        nc.vector.tensor_mul(tmp, osb[0:DH, :], rb)
                        nc.sync.dma_start(
                            out=ot_sb[p][64:128, c * QC:(c + 1) * QC],
                            in_=tmp)

            def gen_av_kb(p, c, holder):
                """attn@V accumulation only (interleaved h01), for the LAST
                unit; the normalize tail is emitted explicitly by the
                scheduler so y-filler can interleave with its latency."""
                pt = pt_tiles.pop((p, c))
                psos = [psO_pool.tile([128, QC], f32, tag="psO",
                                      name=f"psOil{h01}") for h01 in range(2)]
                holder.extend(psos)
                for kb in range(KB):
                    for h01 in range(2):
                        nc.tensor.matmul(
                            psos[h01],
                            lhsT=v_sb[kb][:, p * 2 + h01, :],
                            rhs=pt[:, kb, h01, :],
                            start=(kb == 0), stop=(kb == KB - 1))
                    yield

            def emit_sT_last(p, c, av, selfav):
                """Last unit: previous attn@V first, then own interleaved
                attn@V one slot behind the exp stream."""
                pt = pt_pool.tile([128, KB, 2, QC], bf16, tag="pt",
                                  name=f"pt{p}{c}")
                pt_tiles[(p, c)] = pt
                for kb in range(KB):
                    ps = psS_pool.tile([128, 2, QC], f32, tag="psS",
                                       name=f"psS{p}{c}{kb}")
                    for h01 in range(2):
                        nc.tensor.matmul(
                            ps[:, h01, :],
                            lhsT=kt_sb[p][h01 * 64:(h01 + 1) * 64,
                                          kb * 128:(kb + 1) * 128],
                            rhs=qt_sb[p][h01 * 64:(h01 + 1) * 64,
                                         c * QC:(c + 1) * QC],
                            start=True, stop=True,
                            tile_position=(h01 * 64, 0))
                    nc.scalar.activation(
                        out=pt[:, kb, :, :], in_=ps,
                        func=AF.Exp, scale=0.125)
                    if av is not None and not av.done:
                        av.step()
                        av.step()
                        av.step()
                        av.step()
                    elif kb >= 10:
                        for _ in range(3):
                            if not selfav.step():
                                pump(1)
                    else:
                        pump(3)

            def gen_y(c, scalar_copy=False, qbs=None, alt_pool=False,
                      p_order=None):
                """Output-projection partials for the query blocks of chunk c."""
                p1 = (psO_pool, "psO") if alt_pool else (mm_pool, "mm")
                porder = list(range(PAIRS)) if p_order is None else list(p_order)
                for qb in (range(c * (QC // 128), (c + 1) * (QC // 128))
                           if qbs is None else qbs):
                    yb = yb_pool.tile([128, D], bf16, tag="yb", name=f"yb{qb}")
                    pss = [mm_pool.tile([128, 512], f32, tag="mm",
                                        name=f"psy{qb}0"),
                           p1[0].tile([128, 512], f32, tag=p1[1],
                                      name=f"psy{qb}1")]
                    for pi, p in enumerate(porder):
                        for n in range(2):
                            nc.tensor.matmul(
                                pss[n],
                                lhsT=ot_sb[p][:, qb * 128:(qb + 1) * 128],
                                rhs=wo_sb[p][:, n * 512:(n + 1) * 512],
                                start=(pi == 0), stop=(pi == PAIRS - 1))
                            yield
                    nc.vector.tensor_copy(yb[:, 0:512], pss[0])
                    if scalar_copy:
                        nc.scalar.copy(yb[:, 512:1024], pss[1])
                    else:
                        nc.vector.tensor_copy(yb[:, 512:1024], pss[1])
                    nc.sync.dma_start(out=y_d[qb * 128:(qb + 1) * 128, :], in_=yb)

            # ---- emission schedule ----
            order = [(0, 0), (0, 1), (0, 2), (1, 0), (1, 1), (1, 2),
                     (0, 3), (1, 3)]
            # Deadline-ordered feed per unit.  xt quarter q gates kt/qt
            # chunk q and V blocks 4q..4q+3, so the interleave below only
            # emits work whose data will have landed.
            feed = {
                # unit 1 (0,0): kt0 c1-c3 (slot deadlines 4/8/12), V
                # (attn@V of u1 consumes v_sb from u2 slot 0), qt0 c1 (u2).
                0: [Gen(gen_qk_chunk(1, 0, 1)), Gen(gen_v(0, 2)),
                    Gen(gen_qk_chunk(1, 0, 2)), Gen(gen_v(2, 4)),
                    Gen(gen_qk_chunk(1, 0, 3)), Gen(gen_v(4, 8)),
                    Gen(gen_qk_chunk(0, 0, 1)), Gen(gen_v(8, 15))],
                # unit 2 (0,1): V tail (attn@V h0 reads v_sb[15] at ~slot
                # 7.5; 9 quanta at 2/slot finish by slot 4.5), then qt0 c2
                # (u3 slot 0), qt1 c0 + kt1 c0 (u4).
                1: [Gen(gen_v(15, 16)), Gen(gen_qk_chunk(0, 0, 2)),
                    Gen(gen_qk_chunk(0, 1, 0)), Gen(gen_qk_chunk(1, 1, 0))],
                # unit 3 (0,2): kt1 rest (u4 slots 4/8/12).
                2: [Gen(gen_qk_chunk(1, 1, 1)), Gen(gen_qk_chunk(1, 1, 2)),
                    Gen(gen_qk_chunk(1, 1, 3))],
                # unit 4 (1,0): qt1 c1 (u5).
                3: [Gen(gen_qk_chunk(0, 1, 1))],
                # unit 5 (1,1): qt1 c2 (u6).
                4: [Gen(gen_qk_chunk(0, 1, 2))],
                # unit 6 (1,2): qt0 c3 (u7); y(c0) joins after u5's drain.
                5: [Gen(gen_qk_chunk(0, 0, 3))],
                # unit 7 (0,3): qt1 c3 (u8); y(c1) joins.
                6: [Gen(gen_qk_chunk(0, 1, 3))],
                # unit 8 (1,3): y(c2) first half joins; second half reserved.
                7: [],
            }

            av = None
            prev = None
            y_after = {(1, 0): 0, (1, 1): 1, (1, 2): 2}
            reserved = []
            for ui, (p, c) in enumerate(order):
                pending.extend(feed[ui])
                if ui == len(order) - 1:
                    psos = []
                    selfav = Gen(gen_av_kb(p, c, psos))
                    emit_sT_last(p, c, av, selfav)
                    if av is not None:
                        drain(av)
                    drain(selfav)

                    # Explicit tail: normalize each half with a
                    # PE-broadcast reciprocal (ones[1,64] K=1 matmul
                    # replaces the slow DRAM-replicate bounce) and drain
                    # reserved y(c2) blocks between the DVE steps so the
                    # PE never idles long enough to drop its p-state.
                    ds2 = dram_pool.tile([2, QC], f32, tag="ds2",
                                         name="ds2T")
                    rbs2 = w_pool.tile([64, 2, QC], f32, tag="rbs2",
                                       name="rbs2T")

                    def norm_a(h01):
                        osb = sm_pool.tile([DH + 1, QC], f32, tag="osb",
                                           name=f"osbT{h01}")
                        nc.vector.tensor_copy(osb, psos[h01][0:DH + 1, :])
                        nc.sync.dma_start(out=ds2[h01:h01 + 1, :],
                                          in_=osb[DH:DH + 1, :])
                        return osb, h01

                    def norm_repl():
                        # ONE replicate for both halves: partition-step-0
                        # read of the two denominator rows.
                        dsap = ds2[0:1, 0:1]
                        nc.sync.dma_start(
                            out=rbs2,
                            in_=bass.AP(tensor=dsap.tensor, offset=dsap.offset,
                                        ap=[[0, 64], [QC, 2], [1, QC]]))

                    def norm_b(h01, osb, _):
                        rbp = sm_pool.tile([64, QC], f32, tag="rb",
                                           name=f"rbT{h01}")
                        nc.vector.reciprocal_approx_fast(
                            out=rbp, in_=rbs2[:, h01, :])
                        if h01 == 0:
                            nc.vector.tensor_mul(
                                ot_sb[p][0:64, c * QC:(c + 1) * QC],
                                osb[0:DH, :], rbp)
                        else:
                            tmp = sm_pool.tile([64, QC], bf16, tag="ottmp",
                                               name="ottmpT")
                            nc.vector.tensor_mul(tmp, osb[0:DH, :],
                                                 rbp)
                            nc.sync.dma_start(
                                out=ot_sb[p][64:128, c * QC:(c + 1) * QC],
                                in_=tmp)

                    osb1, _ = norm_a(1)
                    osb0, _ = norm_a(0)
                    norm_repl()
                    if reserved:
                        drain(reserved[0])
                    norm_b(1, osb1, None)
                    for g in reserved[1:]:
                        drain(g)
                    norm_b(0, osb0, None)
                    pump(1 << 30)

                    # Final y: two passes per 2-qb group - pair 0 (whose ot
                    # was normalized during this unit's slots) streams first
                    # so the PE never stalls head-of-line on the pair-1 ot
                    # still in the normalize bounce.
                    qbs = [c * 4 + i for i in range(4)]
                    for gi, half in enumerate((qbs[0:2], qbs[2:4])):
                        tiles = {}
                        for qb in half:
                            yb = yb_pool.tile([128, D], bf16, tag="yb",
                                              name=f"ybF{qb}")
                            pss = [mm_pool.tile([128, 512], f32, tag="mm",
                                                name=f"psyF{qb}0"),
                                   psO_pool.tile([128, 512], f32, tag="psO",
                                                 name=f"psyF{qb}1")]
                            tiles[qb] = (yb, pss)
                        for qb in half:
                            for n in range(2):
                                nc.tensor.matmul(
                                    tiles[qb][1][n],
                                    lhsT=ot_sb[0][:, qb * 128:(qb + 1) * 128],
                                    rhs=wo_sb[0][:, n * 512:(n + 1) * 512],
                                    start=True, stop=False)
                        for qb in half:
                            yb, pss = tiles[qb]
                            for n in range(2):
                                nc.tensor.matmul(
                                    pss[n],
                                    lhsT=ot_sb[1][:, qb * 128:(qb + 1) * 128],
                                    rhs=wo_sb[1][:, n * 512:(n + 1) * 512],
                                    start=False, stop=True)
                            nc.vector.tensor_copy(yb[:, 0:512], pss[0])
                            nc.sync.dma_start(
                                out=y_d[qb * 128:(qb + 1) * 128, 0:512],
                                in_=yb[:, 0:512])
                            nc.scalar.copy(yb[:, 512:1024], pss[1])
                            nc.sync.dma_start(
                                out=y_d[qb * 128:(qb + 1) * 128, 512:1024],
                                in_=yb[:, 512:1024])
                else:
                    emit_sT(p, c, av, base_pump=(11 if prev is None else 4),
                            av_pump=2)
                    if av is not None:
                        drain(av)
                    if prev in y_after:
                        yc = y_after[prev]
                        if yc == 2:
                            pending.append(Gen(gen_y(2, qbs=[8])))
                            for qb in (9, 10, 11):
                                reserved.append(Gen(gen_y(2, qbs=[qb],
                                                          scalar_copy=True,
                                                          alt_pool=True)))
                        else:
                            pending.append(Gen(gen_y(yc)))
                    av = Gen(gen_av(p, c))
                prev = (p, c)
            pump(1 << 30)

    nc.compile()
    return nc


def _get_module(seq=S):
    if seq not in _cache:
        _cache[seq] = _build_module(seq)
    return _cache[seq]


def _wslab(Wq_s, Wk_s, p):
    """[wq pair p | wk pair p] as [128, KT*256] (partition-major: row d%128,
    col k*256 + c) matching the SBUF tile [128, KT, 256]."""
    KT = D // 128
    cols = slice(p * 128, (p + 1) * 128)
    wq_r = Wq_s[:, cols].reshape(KT, 128, 128)
    wk_r = Wk_s[:, cols].reshape(KT, 128, 128)
    w = np.concatenate([wq_r, wk_r], axis=2)      # [KT, 128, 256]
    return np.ascontiguousarray(w.transpose(1, 0, 2).reshape(128, KT * 256))


def _make_in_maps(x, Wq, bq, Wk, bk, Wv, bv, Wo):
    import ml_dtypes
    bf16 = ml_dtypes.bfloat16
    KT = D // 128
    in_maps = []
    for c in range(NCORES):
        b, hg = divmod(c, 4)
        js = slice(hg * J, (hg + 1) * J)
        bqs = np.asarray(bq[js], np.float32)
        bks = np.asarray(bk[js], np.float32)
        bqk = np.stack([bqs[0:128], bqs[128:256],
                        bks[0:128], bks[128:256]], axis=1)
        Wq_s = np.asarray(Wq, np.float32)[:, js]
        Wk_s = np.asarray(Wk, np.float32)[:, js]
        Wv_s = np.asarray(Wv, np.float32)[:, js]
        wv_slab = np.ascontiguousarray(
            Wv_s.reshape(KT, 128, 256).transpose(1, 0, 2).reshape(128, KT * 256))
        bvr = np.broadcast_to(np.asarray(bv[js], np.float32).reshape(1, J),
                              (128, J))
        in_maps.append({
            "xT": np.ascontiguousarray(np.asarray(x[b], np.float32).T).astype(bf16),
            "w0": _wslab(Wq_s, Wk_s, 0).astype(bf16),
            "w1": _wslab(Wq_s, Wk_s, 1).astype(bf16),
            "wv": wv_slab.astype(bf16),
            "wo": np.ascontiguousarray(np.asarray(Wo, np.float32)[js, :]).astype(bf16),
            "bqk": np.ascontiguousarray(bqk.astype(np.float32)),
            "bv": np.ascontiguousarray(bvr).astype(bf16),
        })
    return in_maps


def _gather(results, bo):
    y = np.zeros((B, S, D), np.float32)
    for b in range(B):
        acc = np.zeros((S, D), np.float32)
        for hg in range(4):
            acc += np.asarray(results[b * 4 + hg]["y"], np.float32)
        y[b] = acc + np.asarray(bo, np.float32)[None, :]
    return y


def run_on_hw(inputs, trace=False, **kwargs):
    """Returns (y_full, BassKernelResults)."""
    from concourse.bass_utils import run_bass_kernel_spmd
    nc = _get_module()
    in_maps = _make_in_maps(
        inputs["x"], inputs["Wq"], inputs["bq"], inputs["Wk"], inputs["bk"],
        inputs["Wv"], inputs["bv"], inputs["Wo"])
    res = run_bass_kernel_spmd(nc, in_maps, core_ids=list(range(NCORES)),
                               trace=trace, **kwargs)
    y = _gather(res.results, inputs["bo"])
    return y, res


def kernel(x, Wq, bq, Wk, bk, Wv, bv, Wo, bo):
    y, _ = run_on_hw(dict(x=x, Wq=Wq, bq=bq, Wk=Wk, bk=bk, Wv=Wv, bv=bv,
                          Wo=Wo, bo=bo))
    return y


# revision 32
# speedup vs baseline: 1.0036x; 1.0036x over previous
# Multi-head attention (B=2, S=2048, D=1024, H=16) on 8 TRN2 NeuronCores.
#
# Sharding: core c handles batch b = c//4 and head-group hg = c%4 (4 heads,
# channel slice J = hg*256 : (hg+1)*256).  Each core computes
#   Q^T/K^T = W^T x^T (+bias), V = x W (+bias),
#   S^T_h = K_h^T^T-contraction (d on partitions)  -> exp on ScalarE,
#   O^T_h = [V | 1]^T P^T_h  (row 64 = softmax denominator),
#   y_partial = O^T^T Wo_slice    (bf16, [S, D])
# Host sums the 4 partials per batch and adds bo.
#
# Schedule (v2): the PE stream is the binding path, so everything is
# organized to keep it gapless from ~10us on:
#  - xT and Wqkv live in single SBUF tiles [128, KT, .] so one DMA
#    instruction covers a column-slice of ALL k-tiles (3D DRAM AP).  The
#    pair-0 wq/wk columns load first (0.5MB prefix), then xt streams in
#    seq-quarters; the first scores unit starts at ~14us instead of ~32us.
#  - Weight/bias/wv DMAs ride the Scalar engine's HWDGE queue so the sync
#    engine's descriptor time stays off the critical path.
#  - Unit order (0,0),(0,1),(0,2),(1,0),(1,1),(1,2),(0,3),(1,3): y(c)
#    unlocks after the second pair of chunk c, spreading output DMAs.
#  - Pump generators are drained into the exp slots in deadline order
#    (emission order == execution order per engine); unit 1 pumps hard
#    (PE-paced) to finish V + the remaining kt0 chunks before attn@V of
#    unit 1 consumes them during unit 2.
#  - Tail: the last unit interleaves its own attn@V one slot behind the
#    exp stream; a reserved half of y(c2) covers the final normalize
#    latency so the PE p-state never drops before the last y matmuls.
#
# All matmuls bf16; scores use K=64 tile packing (two heads' MMs run
# CONCURRENT in disjoint PE row groups).  Q/K biases fold into the
# PSUM->SBUF cast (per-partition tensor_scalar add); V bias folds into its
# cast.  Softmax: denominator row 64 bounces through DRAM to replicate
# across partitions, then reciprocal_approx_fast.

import numpy as np

B = 2
S = 2048
D = 1024
H = 16
DH = 64
NCORES = 8
HL = 4            # heads per core
J = HL * DH       # 256: per-core channel slice of D
PAIRS = 2         # head-pairs per core

_cache = {}


def _build_module(seq=S):
    import concourse.bass as bass
    import concourse.mybir as mybir
    import concourse.tile as tile

    from concourse import bacc

    dt = mybir.dt
    f32 = dt.float32
    bf16 = dt.bfloat16
    AF = mybir.ActivationFunctionType

    KB = seq // 128          # key blocks (partition tiles of the key dim)
    QC = min(512, seq)       # query chunk (matmul free dim)
    NQ = seq // QC           # query chunks
    NCH = min(512, seq)      # projection free-dim chunk
    NP = seq // NCH          # projection chunks
    KT = D // 128            # contraction tiles for projections (8)

    nc = bacc.Bacc("TRN2", target_bir_lowering=False, debug=False)

    xT_d = nc.dram_tensor("xT", [D, seq], bf16, kind="ExternalInput").ap()
    # Weight slabs pre-transposed on host to [128, KT*256] (partition-major)
    # so each loads with ONE contiguous 2D DMA: w0/w1 = [wq_p|wk_p] of head
    # pair p, wv = the V weights.
    w0_d = nc.dram_tensor("w0", [128, KT * 256], bf16, kind="ExternalInput").ap()
    w1_d = nc.dram_tensor("w1", [128, KT * 256], bf16, kind="ExternalInput").ap()
    wv_d = nc.dram_tensor("wv", [128, KT * 256], bf16, kind="ExternalInput").ap()
    wo_d = nc.dram_tensor("wo", [J, D], bf16, kind="ExternalInput").ap()
    bqk_d = nc.dram_tensor("bqk", [128, 4], f32, kind="ExternalInput").ap()
    # bv pre-replicated to all 128 partitions on host.
    bv_d = nc.dram_tensor("bv", [128, J], bf16, kind="ExternalInput").ap()
    y_d = nc.dram_tensor("y", [seq, D], bf16, kind="ExternalOutput").ap()

    with tile.TileContext(nc) as tc:
        import contextlib
        ctx = contextlib.ExitStack()
        with ctx:
            xt_pool = ctx.enter_context(tc.tile_pool(name="xt", bufs=1))
            w_pool = ctx.enter_context(tc.tile_pool(name="w", bufs=1))
            qk_pool = ctx.enter_context(tc.tile_pool(name="qk", bufs=1))
            v_pool = ctx.enter_context(tc.tile_pool(name="v", bufs=1))
            pt_pool = ctx.enter_context(tc.tile_pool(name="pt", bufs=2))
            ot_pool = ctx.enter_context(tc.tile_pool(name="ot", bufs=1))
            sm_pool = ctx.enter_context(tc.tile_pool(name="sm", bufs=5))
            yb_pool = ctx.enter_context(tc.tile_pool(name="yb", bufs=4))
            psS_pool = ctx.enter_context(
                tc.tile_pool(name="psS", bufs=2, space="PSUM"))
            psO_pool = ctx.enter_context(
                tc.tile_pool(name="psO", bufs=2, space="PSUM"))
            mm_pool = ctx.enter_context(
                tc.tile_pool(name="mm", bufs=2, space="PSUM"))
            dram_pool = ctx.enter_context(
                tc.tile_pool(name="dscr", bufs=4, space="DRAM"))

            # ---- persistent SBUF tensors ----
            xt_all = xt_pool.tile([128, KT, seq], bf16, tag="xt", name="xt")
            xt_sb = [xt_all[:, k, :] for k in range(KT)]
            # wp_sb[p][:, k, 0:128] = wq pair p, [:, k, 128:256] = wk pair p
            wp_sb = [w_pool.tile([128, KT, 256], bf16, tag=f"w{p}",
                                 name=f"w{p}") for p in range(PAIRS)]
            wv_all = w_pool.tile([128, KT, 256], bf16, tag="wv", name="wv")
            wv_sb = [wv_all[:, k, :] for k in range(KT)]
            wo_sb = [w_pool.tile([128, D], bf16, tag=f"wo{p}",
                                 name=f"wo{p}") for p in range(PAIRS)]
            bqk_sb = w_pool.tile([128, 4], f32, tag="bqk", name="bqk")
            bvr_sb = w_pool.tile([128, J], bf16, tag="bvr", name="bvr")
            dz_sb = w_pool.tile([128, 512], bf16, tag="dz", name="dz")

            qt_sb = [qk_pool.tile([128, seq], bf16, tag=f"qt{p}",
                                  name=f"qt{p}") for p in range(PAIRS)]
            kt_sb = [qk_pool.tile([128, seq], bf16, tag=f"kt{p}",
                                  name=f"kt{p}") for p in range(PAIRS)]
            # V padded to 128 columns per head (NumWeights==128 -> FWL).
            v_sb = [v_pool.tile([128, HL, 128], bf16, tag=f"v{s}",
                                name=f"v{s}") for s in range(KB)]
            ot_sb = [ot_pool.tile([128, seq], bf16, tag=f"ot{p}",
                                  name=f"ot{p}") for p in range(PAIRS)]

            # ---- input DMAs ----
            def xtq(q, k0, k1):
                """One DMA for xt quarter q, k-tiles k0..k1-1."""
                x0 = xT_d[0:1, 0:1]
                nc.sync.dma_start(
                    out=xt_all[:, k0:k1, q * QC:(q + 1) * QC],
                    in_=bass.AP(tensor=x0.tensor,
                                offset=x0.offset + k0 * 128 * seq + q * QC,
                                ap=[[seq, 128], [128 * seq, k1 - k0],
                                    [1, QC]]))

            # Sync queue, deadline order: pair-0 weights + xt quarter 0
            # (gate the prefix chains), bqk (prefix bias), wv + bv (V
            # blocks from ~slot 2), then the later quarters and weights.
            # Sync HWDGE queue (one queue sustains only ~100-150 GB/s
            # serially): the prefix w0 + xt stream in deadline order.  The
            # late-needed weights (w1, wo: ~1MB) ride the gpsimd SWDGE
            # queue in parallel so they don't delay the xt quarters.
            nc.sync.dma_start(out=wp_sb[0], in_=w0_d)
            for kp in range(KT // 2):
                xtq(0, 2 * kp, 2 * kp + 2)
            nc.sync.dma_start(out=bqk_sb, in_=bqk_d)
            nc.sync.dma_start(out=wv_all, in_=wv_d)
            nc.sync.dma_start(out=bvr_sb, in_=bv_d)
            for kp in range(KT // 2):
                xtq(1, 2 * kp, 2 * kp + 2)
            nc.sync.dma_start(out=wp_sb[1], in_=w1_d)
            xtq(2, 0, KT // 2)
            xtq(2, KT // 2, KT)
            for pp in range(PAIRS):
                nc.sync.dma_start(out=wo_sb[pp],
                                  in_=wo_d[pp * 128:(pp + 1) * 128, :])
            xtq(3, 0, KT // 2)
            xtq(3, KT // 2, KT)

            # PE p-state warm-up: dummy matmuls on a zeroed tile ramp the
            # tensor clock while the first DMAs land.
            nc.vector.memset(dz_sb, 0.0)
            trash = psS_pool.tile([128, 2, QC], f32, tag="psS", name="trash")
            for i in range(14):
                nc.tensor.matmul(trash[:, 0, :], lhsT=dz_sb[:, 0:128],
                                 rhs=dz_sb, start=True, stop=True)
            for s in range(KB):
                nc.vector.memset(v_sb[s][:, :, DH:DH + 1], 1.0)
                nc.vector.memset(v_sb[s][:, :, DH + 1:], 0.0)
            # Warm the exp table set during the DMA ramp.
            warm = w_pool.tile([1, 8], f32, tag="warm", name="warm")
            nc.vector.memset(warm, 0.0)
            nc.scalar.activation(out=warm, in_=warm, func=AF.Exp)

            def wq_s(p, k):
                return wp_sb[p][:, k, 0:128]

            def wk_s(p, k):
                return wp_sb[p][:, k, 128:256]

            # ---- prefix: kt0 chunk 0 + qt0 chunk 0, k-outer (DMA-paced) ----
            ps_k0 = mm_pool.tile([128, 512], f32, tag="mm", name="pfx_k")
            ps_q0 = mm_pool.tile([128, 512], f32, tag="mm", name="pfx_q")
            for k in range(KT):
                nc.tensor.matmul(ps_k0[:, :NCH], lhsT=wk_s(0, k),
                                 rhs=xt_sb[k][:, 0:NCH],
                                 start=(k == 0), stop=(k == KT - 1))
                nc.tensor.matmul(ps_q0[:, :NCH], lhsT=wq_s(0, k),
                                 rhs=xt_sb[k][:, 0:NCH],
                                 start=(k == 0), stop=(k == KT - 1))
            nc.vector.tensor_scalar_add(kt_sb[0][:, 0:NCH], ps_k0[:, :NCH],
                                        bqk_sb[:, 2:3])
            nc.vector.tensor_scalar_add(qt_sb[0][:, 0:NCH], ps_q0[:, :NCH],
                                        bqk_sb[:, 0:1])

            # ---- emission helpers ----
            def gen_qk_chunk(which, p, nck):
                """One Q^T (which=0) / K^T (which=1) chunk, k-inner."""
                w_f = wq_s if which == 0 else wk_s
                dst = qt_sb[p] if which == 0 else kt_sb[p]
                bcol = which * 2 + p
                ps = mm_pool.tile([128, 512], f32, tag="mm",
                                  name=f"psqk{which}{p}{nck}")
                for k in range(KT):
                    nc.tensor.matmul(
                        ps[:, :NCH],
                        lhsT=w_f(p, k),
                        rhs=xt_sb[k][:, nck * NCH:(nck + 1) * NCH],
                        start=(k == 0), stop=(k == KT - 1))
                    yield
                nc.vector.tensor_scalar_add(
                    dst[:, nck * NCH:(nck + 1) * NCH], ps[:, :NCH],
                    bqk_sb[:, bcol:bcol + 1])
                yield

            def gen_v(s0, s1):
                """V blocks s0..s1-1; bias added during the psum cast."""
                for s in range(s0, s1):
                    ps = mm_pool.tile([128, 512], f32, tag="mm", name=f"psv{s}")
                    for k in range(KT):
                        nc.tensor.matmul(
                            ps[:, :J],
                            lhsT=xt_sb[k][:, s * 128:(s + 1) * 128],
                            rhs=wv_sb[k],
                            start=(k == 0), stop=(k == KT - 1))
                        yield
                    nc.vector.tensor_add(
                        v_sb[s][:, :, 0:DH],
                        ps[:, :J].rearrange("p (h d) -> p h d", h=HL),
                        bvr_sb.rearrange("p (h d) -> p h d", h=HL))
                    yield

            pt_tiles = {}

            class Gen:
                def __init__(self, it):
                    self.it = it
                    self.done = False

                def step(self):
                    if self.done:
                        return False
                    try:
                        next(self.it)
                        return True
                    except StopIteration:
                        self.done = True
                        return False

            pending = []

            def pump(n):
                while n > 0 and pending:
                    if pending[0].step():
                        n -= 1
                    else:
                        pending.pop(0)

            def drain(g):
                while g.step():
                    pass

            def emit_sT(p, c, av, base_pump=4, av_pump=2):
                """Scores^T + exp for head-pair p, query chunk c."""
                pt = pt_pool.tile([128, KB, 2, QC], bf16, tag="pt",
                                  name=f"pt{p}{c}")
                pt_tiles[(p, c)] = pt
                for kb in range(KB):
                    ps = psS_pool.tile([128, 2, QC], f32, tag="psS",
                                       name=f"psS{p}{c}{kb}")
                    for h01 in range(2):
                        nc.tensor.matmul(
                            ps[:, h01, :],
                            lhsT=kt_sb[p][h01 * 64:(h01 + 1) * 64,
                                          kb * 128:(kb + 1) * 128],
                            rhs=qt_sb[p][h01 * 64:(h01 + 1) * 64,
                                         c * QC:(c + 1) * QC],
                            start=True, stop=True,
                            tile_position=(h01 * 64, 0))
                    nc.scalar.activation(
                        out=pt[:, kb, :, :], in_=ps,
                        func=AF.Exp, scale=0.125)
                    if av is not None and not av.done:
                        av.step()
                        av.step()
                        pump(av_pump)
                    else:
                        pump(base_pump)

            def gen_av(p, c):
                """attn @ [V|1], reciprocal, normalize, build O^T pair tile."""
                pt = pt_tiles.pop((p, c))
                for h01 in range(2):
                    h = p * 2 + h01
                    pso = psO_pool.tile([128, QC], f32, tag="psO",
                                        name=f"psO{p}{c}{h01}")
                    for kb in range(KB):
                        nc.tensor.matmul(
                            pso,
                            lhsT=v_sb[kb][:, h, :],
                            rhs=pt[:, kb, h01, :],
                            start=(kb == 0), stop=(kb == KB - 1))
                        yield
                    osb = sm_pool.tile([DH + 1, QC], f32, tag="osb",
                                       name=f"osb{p}{c}{h01}")
                    nc.vector.tensor_copy(osb, pso[0:DH + 1, :])
                    yield
                    ds = dram_pool.tile([1, QC], f32, tag="ds",
                                        name=f"ds{p}{c}{h01}")
                    nc.sync.dma_start(out=ds, in_=osb[DH:DH + 1, :])
                    dsap = ds[0:1, :]
                    rbs = sm_pool.tile([64, QC], f32, tag="rbs",
                                       name=f"rbs{p}{c}{h01}")
                    nc.sync.dma_start(
                        out=rbs,
                        in_=bass.AP(tensor=dsap.tensor, offset=dsap.offset,
                                    ap=[[0, 64], [1, QC]]))
                    rb = sm_pool.tile([64, QC], f32, tag="rb",
                                      name=f"rb{p}{c}{h01}")
                    nc.vector.reciprocal_approx_fast(out=rb, in_=rbs)
                    if h01 == 0:
                        nc.vector.tensor_mul(
                            ot_sb[p][0:64, c * QC:(c + 1) * QC],
                            osb[0:DH, :], rb)
                    else:
                        tmp = sm_pool.tile([64, QC], bf16, tag="ottmp",
                                           name=f"ottmp{p}{c}")
                        nc.vector.tensor_mul(tmp, osb[0:DH, :], rb)
                        nc.sync.dma_start(
                            out=ot_sb[p][64:128, c * QC:(c + 1) * QC],
                            in_=tmp)

            def gen_av_kb(p, c, holder):
                """attn@V accumulation only (interleaved h01), for the LAST
                unit; the normalize tail is emitted explicitly by the
                scheduler so y-filler can interleave with its latency."""
                pt = pt_tiles.pop((p, c))
                psos = [psO_pool.tile([128, QC], f32, tag="psO",
                                      name=f"psOil{h01}") for h01 in range(2)]
                holder.extend(psos)
                for kb in range(KB):
                    for h01 in range(2):
                        nc.tensor.matmul(
                            psos[h01],
                            lhsT=v_sb[kb][:, p * 2 + h01, :],
                            rhs=pt[:, kb, h01, :],
                            start=(kb == 0), stop=(kb == KB - 1))
                    yield

            def emit_sT_last(p, c, av, selfav):
                """Last unit: previous attn@V first, then own interleaved
                attn@V one slot behind the exp stream."""
                pt = pt_pool.tile([128, KB, 2, QC], bf16, tag="pt",
                                  name=f"pt{p}{c}")
                pt_tiles[(p, c)] = pt
                for kb in range(KB):
                    ps = psS_pool.tile([128, 2, QC], f32, tag="psS",
                                       name=f"psS{p}{c}{kb}")
                    for h01 in range(2):
                        nc.tensor.matmul(
                            ps[:, h01, :],
                            lhsT=kt_sb[p][h01 * 64:(h01 + 1) * 64,
                                          kb * 128:(kb + 1) * 128],
                            rhs=qt_sb[p][h01 * 64:(h01 + 1) * 64,
                                         c * QC:(c + 1) * QC],
                            start=True, stop=True,
                            tile_position=(h01 * 64, 0))
                    nc.scalar.activation(
                        out=pt[:, kb, :, :], in_=ps,
                        func=AF.Exp, scale=0.125)
                    if av is not None and not av.done:
                        av.step()
                        av.step()
                        av.step()
                        av.step()
                    elif kb >= 10:
                        for _ in range(3):
                            if not selfav.step():
                                pump(1)
                    else:
                        pump(3)

            def gen_y(c, scalar_copy=False, qbs=None, alt_pool=False,
                      p_order=None):
                """Output-projection partials for the query blocks of chunk c."""
                p1 = (psO_pool, "psO") if alt_pool else (mm_pool, "mm")
                porder = list(range(PAIRS)) if p_order is None else list(p_order)
                for qb in (range(c * (QC // 128), (c + 1) * (QC // 128))
                           if qbs is None else qbs):
                    yb = yb_pool.tile([128, D], bf16, tag="yb", name=f"yb{qb}")
                    pss = [mm_pool.tile([128, 512], f32, tag="mm",
                                        name=f"psy{qb}0"),
                           p1[0].tile([128, 512], f32, tag=p1[1],
                                      name=f"psy{qb}1")]
                    for pi, p in enumerate(porder):
                        for n in range(2):
                            nc.tensor.matmul(
                                pss[n],
                                lhsT=ot_sb[p][:, qb * 128:(qb + 1) * 128],
                                rhs=wo_sb[p][:, n * 512:(n + 1) * 512],
                                start=(pi == 0), stop=(pi == PAIRS - 1))
                            yield
                    nc.vector.tensor_copy(yb[:, 0:512], pss[0])
                    if scalar_copy:
                        nc.scalar.copy(yb[:, 512:1024], pss[1])
                    else:
                        nc.vector.tensor_copy(yb[:, 512:1024], pss[1])
                    nc.sync.dma_start(out=y_d[qb * 128:(qb + 1) * 128, :], in_=yb)

            # ---- emission schedule ----
            order = [(0, 0), (0, 1), (0, 2), (1, 0), (1, 1), (1, 2),
                     (0, 3), (1, 3)]
            # Deadline-ordered feed per unit.  xt quarter q gates kt/qt
            # chunk q and V blocks 4q..4q+3, so the interleave below only
            # emits work whose data will have landed.
            feed = {
                # unit 1 (0,0): kt0 c1-c3 (slot deadlines 4/8/12), V
                # (attn@V of u1 consumes v_sb from u2 slot 0), qt0 c1 (u2).
                0: [Gen(gen_qk_chunk(1, 0, 1)), Gen(gen_v(0, 2)),
                    Gen(gen_qk_chunk(1, 0, 2)), Gen(gen_v(2, 4)),
                    Gen(gen_qk_chunk(1, 0, 3)), Gen(gen_v(4, 8)),
                    Gen(gen_qk_chunk(0, 0, 1)), Gen(gen_v(8, 15))],
                # unit 2 (0,1): V tail (attn@V h0 reads v_sb[15] at ~slot
                # 7.5; 9 quanta at 2/slot finish by slot 4.5), then qt0 c2
                # (u3 slot 0), qt1 c0 + kt1 c0 (u4).
                1: [Gen(gen_v(15, 16)), Gen(gen_qk_chunk(0, 0, 2)),
                    Gen(gen_qk_chunk(0, 1, 0)), Gen(gen_qk_chunk(1, 1, 0))],
                # unit 3 (0,2): kt1 rest (u4 slots 4/8/12).
                2: [Gen(gen_qk_chunk(1, 1, 1)), Gen(gen_qk_chunk(1, 1, 2)),
                    Gen(gen_qk_chunk(1, 1, 3))],
                # unit 4 (1,0): qt1 c1 (u5).
                3: [Gen(gen_qk_chunk(0, 1, 1))],
                # unit 5 (1,1): qt1 c2 (u6).
                4: [Gen(gen_qk_chunk(0, 1, 2))],
                # unit 6 (1,2): qt0 c3 (u7); y(c0) joins after u5's drain.
                5: [Gen(gen_qk_chunk(0, 0, 3))],
                # unit 7 (0,3): qt1 c3 (u8); y(c1) joins.
                6: [Gen(gen_qk_chunk(0, 1, 3))],
                # unit 8 (1,3): y(c2) first half joins; second half reserved.
                7: [],
            }

            av = None
            prev = None
            y_after = {(1, 0): 0, (1, 1): 1, (1, 2): 2}
            reserved = []
            for ui, (p, c) in enumerate(order):
                pending.extend(feed[ui])
                if ui == len(order) - 1:
                    psos = []
                    selfav = Gen(gen_av_kb(p, c, psos))
                    emit_sT_last(p, c, av, selfav)
                    if av is not None:
                        drain(av)
                    drain(selfav)

                    # Explicit tail: normalize each half with a
                    # PE-broadcast reciprocal (ones[1,64] K=1 matmul
                    # replaces the slow DRAM-replicate bounce) and drain
                    # reserved y(c2) blocks between the DVE steps so the
                    # PE never idles long enough to drop its p-state.
                    def norm_a(h01):
                        osb = sm_pool.tile([DH + 1, QC], f32, tag="osb",
                                           name=f"osbT{h01}")
                        nc.vector.tensor_copy(osb, psos[h01][0:DH + 1, :])
                        ds = dram_pool.tile([1, QC], f32, tag="ds",
                                            name=f"dsT{h01}")
                        nc.sync.dma_start(out=ds, in_=osb[DH:DH + 1, :])
                        dsap = ds[0:1, :]
                        rbs = sm_pool.tile([64, QC], f32, tag="rbs",
                                           name=f"rbsT{h01}")
                        nc.sync.dma_start(
                            out=rbs,
                            in_=bass.AP(tensor=dsap.tensor, offset=dsap.offset,
                                        ap=[[0, 64], [1, QC]]))
                        return osb, rbs

                    def norm_b(h01, osb, rbs):
                        rbp = sm_pool.tile([64, QC], f32, tag="rb",
                                           name=f"rbT{h01}")
                        nc.vector.reciprocal_approx_fast(out=rbp, in_=rbs)
                        if h01 == 0:
                            nc.vector.tensor_mul(
                                ot_sb[p][0:64, c * QC:(c + 1) * QC],
                                osb[0:DH, :], rbp)
                        else:
                            tmp = sm_pool.tile([64, QC], bf16, tag="ottmp",
                                               name="ottmpT")
                            nc.vector.tensor_mul(tmp, osb[0:DH, :],
                                                 rbp)
                            nc.sync.dma_start(
                                out=ot_sb[p][64:128, c * QC:(c + 1) * QC],
                                in_=tmp)

                    osb1, rbs1 = norm_a(1)
                    if reserved:
                        drain(reserved[0])
                    norm_b(1, osb1, rbs1)
                    osb0, rbs0 = norm_a(0)
                    for g in reserved[1:]:
                        drain(g)
                    norm_b(0, osb0, rbs0)
                    pump(1 << 30)

                    # Final y: two passes per 2-qb group - pair 0 (whose ot
                    # was normalized during this unit's slots) streams first
                    # so the PE never stalls head-of-line on the pair-1 ot
                    # still in the normalize bounce.
                    qbs = [c * 4 + i for i in range(4)]
                    for gi, half in enumerate((qbs[0:2], qbs[2:4])):
                        tiles = {}
                        for qb in half:
                            yb = yb_pool.tile([128, D], bf16, tag="yb",
                                              name=f"ybF{qb}")
                            pss = [mm_pool.tile([128, 512], f32, tag="mm",
                                                name=f"psyF{qb}0"),
                                   psO_pool.tile([128, 512], f32, tag="psO",
                                                 name=f"psyF{qb}1")]
                            tiles[qb] = (yb, pss)
                        for qb in half:
                            for n in range(2):
                                nc.tensor.matmul(
                                    tiles[qb][1][n],
                                    lhsT=ot_sb[0][:, qb * 128:(qb + 1) * 128],
                                    rhs=wo_sb[0][:, n * 512:(n + 1) * 512],
                                    start=True, stop=False)
                        for qb in half:
                            yb, pss = tiles[qb]
                            for n in range(2):
                                nc.tensor.matmul(
                                    pss[n],
                                    lhsT=ot_sb[1][:, qb * 128:(qb + 1) * 128],
                                    rhs=wo_sb[1][:, n * 512:(n + 1) * 512],
                                    start=False, stop=True)
                            nc.vector.tensor_copy(yb[:, 0:512], pss[0])
                            nc.sync.dma_start(
                                out=y_d[qb * 128:(qb + 1) * 128, 0:512],
                                in_=yb[:, 0:512])
                            nc.scalar.copy(yb[:, 512:1024], pss[1])
                            nc.sync.dma_start(
                                out=y_d[qb * 128:(qb + 1) * 128, 512:1024],
                                in_=yb[:, 512:1024])
                else:
                    emit_sT(p, c, av, base_pump=(11 if prev is None else 4),
                            av_pump=2)
                    if av is not None:
                        drain(av)
                    if prev in y_after:
                        yc = y_after[prev]
                        if yc == 2:
                            pending.append(Gen(gen_y(2, qbs=[8])))
                            for qb in (9, 10, 11):
                                reserved.append(Gen(gen_y(2, qbs=[qb],
                                                          scalar_copy=True,
                                                          alt_pool=True)))
                        else:
                            pending.append(Gen(gen_y(yc)))
                    av = Gen(gen_av(p, c))
                prev = (p, c)
            pump(1 << 30)

    nc.compile()
    return nc


def _get_module(seq=S):
    if seq not in _cache:
        _cache[seq] = _build_module(seq)
    return _cache[seq]


def _wslab(Wq_s, Wk_s, p):
    """[wq pair p | wk pair p] as [128, KT*256] (partition-major: row d%128,
    col k*256 + c) matching the SBUF tile [128, KT, 256]."""
    KT = D // 128
    cols = slice(p * 128, (p + 1) * 128)
    wq_r = Wq_s[:, cols].reshape(KT, 128, 128)
    wk_r = Wk_s[:, cols].reshape(KT, 128, 128)
    w = np.concatenate([wq_r, wk_r], axis=2)      # [KT, 128, 256]
    return np.ascontiguousarray(w.transpose(1, 0, 2).reshape(128, KT * 256))


def _make_in_maps(x, Wq, bq, Wk, bk, Wv, bv, Wo):
    import ml_dtypes
    bf16 = ml_dtypes.bfloat16
    KT = D // 128
    in_maps = []
    for c in range(NCORES):
        b, hg = divmod(c, 4)
        js = slice(hg * J, (hg + 1) * J)
        bqs = np.asarray(bq[js], np.float32)
        bks = np.asarray(bk[js], np.float32)
        bqk = np.stack([bqs[0:128], bqs[128:256],
                        bks[0:128], bks[128:256]], axis=1)
        Wq_s = np.asarray(Wq, np.float32)[:, js]
        Wk_s = np.asarray(Wk, np.float32)[:, js]
        Wv_s = np.asarray(Wv, np.float32)[:, js]
        wv_slab = np.ascontiguousarray(
            Wv_s.reshape(KT, 128, 256).transpose(1, 0, 2).reshape(128, KT * 256))
        bvr = np.broadcast_to(np.asarray(bv[js], np.float32).reshape(1, J),
                              (128, J))
        in_maps.append({
            "xT": np.ascontiguousarray(np.asarray(x[b], np.float32).T).astype(bf16),
            "w0": _wslab(Wq_s, Wk_s, 0).astype(bf16),
            "w1": _wslab(Wq_s, Wk_s, 1).astype(bf16),
            "wv": wv_slab.astype(bf16),
            "wo": np.ascontiguousarray(np.asarray(Wo, np.float32)[js, :]).astype(bf16),
            "bqk": np.ascontiguousarray(bqk.astype(np.float32)),
            "bv": np.ascontiguousarray(bvr).astype(bf16),
        })
    return in_maps


def _gather(results, bo):
    y = np.zeros((B, S, D), np.float32)
    for b in range(B):
        acc = np.zeros((S, D), np.float32)
        for hg in range(4):
            acc += np.asarray(results[b * 4 + hg]["y"], np.float32)
        y[b] = acc + np.asarray(bo, np.float32)[None, :]
    return y


def run_on_hw(inputs, trace=False, **kwargs):
    """Returns (y_full, BassKernelResults)."""
    from concourse.bass_utils import run_bass_kernel_spmd
    nc = _get_module()
    in_maps = _make_in_maps(
        inputs["x"], inputs["Wq"], inputs["bq"], inputs["Wk"], inputs["bk"],
        inputs["Wv"], inputs["bv"], inputs["Wo"])
    res = run_bass_kernel_spmd(nc, in_maps, core_ids=list(range(NCORES)),
                               trace=trace, **kwargs)
    y = _gather(res.results, inputs["bo"])
    return y, res


def kernel(x, Wq, bq, Wk, bk, Wv, bv, Wo, bo):
    y, _ = run_on_hw(dict(x=x, Wq=Wq, bq=bq, Wk=Wk, bk=bk, Wv=Wv, bv=bv,
                          Wo=Wo, bo=bo))
    return y


# revision 33
# speedup vs baseline: 1.0286x; 1.0249x over previous
# Multi-head attention (B=2, S=2048, D=1024, H=16) on 8 TRN2 NeuronCores.
#
# Sharding: core c handles batch b = c//4 and head-group hg = c%4 (4 heads,
# channel slice J = hg*256 : (hg+1)*256).  Each core computes
#   Q^T/K^T = W^T x^T (+bias), V = x W (+bias),
#   S^T_h = K_h^T^T-contraction (d on partitions)  -> exp on ScalarE,
#   O^T_h = [V | 1]^T P^T_h  (row 64 = softmax denominator),
#   y_partial = O^T^T Wo_slice    (bf16, [S, D])
# Host sums the 4 partials per batch and adds bo.
#
# Schedule (v2): the PE stream is the binding path, so everything is
# organized to keep it gapless from ~10us on:
#  - xT and Wqkv live in single SBUF tiles [128, KT, .] so one DMA
#    instruction covers a column-slice of ALL k-tiles (3D DRAM AP).  The
#    pair-0 wq/wk columns load first (0.5MB prefix), then xt streams in
#    seq-quarters; the first scores unit starts at ~14us instead of ~32us.
#  - Weight/bias/wv DMAs ride the Scalar engine's HWDGE queue so the sync
#    engine's descriptor time stays off the critical path.
#  - Unit order (0,0),(0,1),(0,2),(1,0),(1,1),(1,2),(0,3),(1,3): y(c)
#    unlocks after the second pair of chunk c, spreading output DMAs.
#  - Pump generators are drained into the exp slots in deadline order
#    (emission order == execution order per engine); unit 1 pumps hard
#    (PE-paced) to finish V + the remaining kt0 chunks before attn@V of
#    unit 1 consumes them during unit 2.
#  - Tail: the last unit interleaves its own attn@V one slot behind the
#    exp stream; a reserved half of y(c2) covers the final normalize
#    latency so the PE p-state never drops before the last y matmuls.
#
# All matmuls bf16; scores use K=64 tile packing (two heads' MMs run
# CONCURRENT in disjoint PE row groups).  Q/K biases fold into the
# PSUM->SBUF cast (per-partition tensor_scalar add); V bias folds into its
# cast.  Softmax: denominator row 64 bounces through DRAM to replicate
# across partitions, then reciprocal_approx_fast.

import numpy as np

B = 2
S = 2048
D = 1024
H = 16
DH = 64
NCORES = 8
HL = 4            # heads per core
J = HL * DH       # 256: per-core channel slice of D
PAIRS = 2         # head-pairs per core

_cache = {}


def _build_module(seq=S):
    import concourse.bass as bass
    import concourse.mybir as mybir
    import concourse.tile as tile

    from concourse import bacc

    dt = mybir.dt
    f32 = dt.float32
    bf16 = dt.bfloat16
    AF = mybir.ActivationFunctionType

    KB = seq // 128          # key blocks (partition tiles of the key dim)
    QC = min(512, seq)       # query chunk (matmul free dim)
    NQ = seq // QC           # query chunks
    NCH = min(512, seq)      # projection free-dim chunk
    NP = seq // NCH          # projection chunks
    KT = D // 128            # contraction tiles for projections (8)

    nc = bacc.Bacc("TRN2", target_bir_lowering=False, debug=False)

    xT_d = nc.dram_tensor("xT", [D, seq], bf16, kind="ExternalInput").ap()
    # Weight slabs pre-transposed on host to [128, KT*256] (partition-major)
    # so each loads with ONE contiguous 2D DMA: w0/w1 = [wq_p|wk_p] of head
    # pair p, wv = the V weights.
    w0_d = nc.dram_tensor("w0", [128, KT * 256], bf16, kind="ExternalInput").ap()
    w1_d = nc.dram_tensor("w1", [128, KT * 256], bf16, kind="ExternalInput").ap()
    wv_d = nc.dram_tensor("wv", [128, KT * 256], bf16, kind="ExternalInput").ap()
    wo_d = nc.dram_tensor("wo", [J, D], bf16, kind="ExternalInput").ap()
    bqk_d = nc.dram_tensor("bqk", [128, 4], f32, kind="ExternalInput").ap()
    # bv pre-replicated to all 128 partitions on host.
    bv_d = nc.dram_tensor("bv", [128, J], bf16, kind="ExternalInput").ap()
    y_d = nc.dram_tensor("y", [seq, D], bf16, kind="ExternalOutput").ap()

    with tile.TileContext(nc) as tc:
        import contextlib
        ctx = contextlib.ExitStack()
        with ctx:
            xt_pool = ctx.enter_context(tc.tile_pool(name="xt", bufs=1))
            w_pool = ctx.enter_context(tc.tile_pool(name="w", bufs=1))
            qk_pool = ctx.enter_context(tc.tile_pool(name="qk", bufs=1))
            v_pool = ctx.enter_context(tc.tile_pool(name="v", bufs=1))
            pt_pool = ctx.enter_context(tc.tile_pool(name="pt", bufs=2))
            ot_pool = ctx.enter_context(tc.tile_pool(name="ot", bufs=1))
            sm_pool = ctx.enter_context(tc.tile_pool(name="sm", bufs=5))
            yb_pool = ctx.enter_context(tc.tile_pool(name="yb", bufs=4))
            psS_pool = ctx.enter_context(
                tc.tile_pool(name="psS", bufs=2, space="PSUM"))
            psO_pool = ctx.enter_context(
                tc.tile_pool(name="psO", bufs=2, space="PSUM"))
            mm_pool = ctx.enter_context(
                tc.tile_pool(name="mm", bufs=2, space="PSUM"))
            dram_pool = ctx.enter_context(
                tc.tile_pool(name="dscr", bufs=4, space="DRAM"))

            # ---- persistent SBUF tensors ----
            xt_all = xt_pool.tile([128, KT, seq], bf16, tag="xt", name="xt")
            xt_sb = [xt_all[:, k, :] for k in range(KT)]
            # wp_sb[p][:, k, 0:128] = wq pair p, [:, k, 128:256] = wk pair p
            wp_sb = [w_pool.tile([128, KT, 256], bf16, tag=f"w{p}",
                                 name=f"w{p}") for p in range(PAIRS)]
            wv_all = w_pool.tile([128, KT, 256], bf16, tag="wv", name="wv")
            wv_sb = [wv_all[:, k, :] for k in range(KT)]
            wo_sb = [w_pool.tile([128, D], bf16, tag=f"wo{p}",
                                 name=f"wo{p}") for p in range(PAIRS)]
            bqk_sb = w_pool.tile([128, 4], f32, tag="bqk", name="bqk")
            bvr_sb = w_pool.tile([128, J], bf16, tag="bvr", name="bvr")
            dz_sb = w_pool.tile([128, 512], bf16, tag="dz", name="dz")

            qt_sb = [qk_pool.tile([128, seq], bf16, tag=f"qt{p}",
                                  name=f"qt{p}") for p in range(PAIRS)]
            kt_sb = [qk_pool.tile([128, seq], bf16, tag=f"kt{p}",
                                  name=f"kt{p}") for p in range(PAIRS)]
            # V padded to 128 columns per head (NumWeights==128 -> FWL).
            v_sb = [v_pool.tile([128, HL, 128], bf16, tag=f"v{s}",
                                name=f"v{s}") for s in range(KB)]
            ot_sb = [ot_pool.tile([128, seq], bf16, tag=f"ot{p}",
                                  name=f"ot{p}") for p in range(PAIRS)]

            # ---- input DMAs ----
            def xtq(q, k0, k1):
                """One DMA for xt quarter q, k-tiles k0..k1-1."""
                x0 = xT_d[0:1, 0:1]
                nc.sync.dma_start(
                    out=xt_all[:, k0:k1, q * QC:(q + 1) * QC],
                    in_=bass.AP(tensor=x0.tensor,
                                offset=x0.offset + k0 * 128 * seq + q * QC,
                                ap=[[seq, 128], [128 * seq, k1 - k0],
                                    [1, QC]]))

            # Sync queue, deadline order: pair-0 weights + xt quarter 0
            # (gate the prefix chains), bqk (prefix bias), wv + bv (V
            # blocks from ~slot 2), then the later quarters and weights.
            # Sync HWDGE queue (one queue sustains only ~100-150 GB/s
            # serially): the prefix w0 + xt stream in deadline order.  The
            # late-needed weights (w1, wo: ~1MB) ride the gpsimd SWDGE
            # queue in parallel so they don't delay the xt quarters.
            nc.sync.dma_start(out=wp_sb[0], in_=w0_d)
            for kp in range(KT // 2):
                xtq(0, 2 * kp, 2 * kp + 2)
            nc.sync.dma_start(out=bqk_sb, in_=bqk_d)
            nc.sync.dma_start(out=wv_all, in_=wv_d)
            nc.sync.dma_start(out=bvr_sb, in_=bv_d)
            for kp in range(KT // 2):
                xtq(1, 2 * kp, 2 * kp + 2)
            nc.sync.dma_start(out=wp_sb[1], in_=w1_d)
            xtq(2, 0, KT // 2)
            xtq(2, KT // 2, KT)
            for pp in range(PAIRS):
                nc.sync.dma_start(out=wo_sb[pp],
                                  in_=wo_d[pp * 128:(pp + 1) * 128, :])
            xtq(3, 0, KT // 2)
            xtq(3, KT // 2, KT)

            # PE p-state warm-up: dummy matmuls on a zeroed tile ramp the
            # tensor clock while the first DMAs land.
            nc.vector.memset(dz_sb, 0.0)
            trash = psS_pool.tile([128, 2, QC], f32, tag="psS", name="trash")
            for i in range(14):
                nc.tensor.matmul(trash[:, 0, :], lhsT=dz_sb[:, 0:128],
                                 rhs=dz_sb, start=True, stop=True)
            for s in range(KB):
                nc.vector.memset(v_sb[s][:, :, DH:DH + 1], 1.0)
                nc.vector.memset(v_sb[s][:, :, DH + 1:], 0.0)
            # Warm the exp table set during the DMA ramp.
            warm = w_pool.tile([1, 8], f32, tag="warm", name="warm")
            nc.vector.memset(warm, 0.0)
            nc.scalar.activation(out=warm, in_=warm, func=AF.Exp)

            def wq_s(p, k):
                return wp_sb[p][:, k, 0:128]

            def wk_s(p, k):
                return wp_sb[p][:, k, 128:256]

            # ---- prefix: kt0 chunk 0 + qt0 chunk 0, k-outer (DMA-paced) ----
            ps_k0 = mm_pool.tile([128, 512], f32, tag="mm", name="pfx_k")
            ps_q0 = mm_pool.tile([128, 512], f32, tag="mm", name="pfx_q")
            for k in range(KT):
                nc.tensor.matmul(ps_k0[:, :NCH], lhsT=wk_s(0, k),
                                 rhs=xt_sb[k][:, 0:NCH],
                                 start=(k == 0), stop=(k == KT - 1))
                nc.tensor.matmul(ps_q0[:, :NCH], lhsT=wq_s(0, k),
                                 rhs=xt_sb[k][:, 0:NCH],
                                 start=(k == 0), stop=(k == KT - 1))
            nc.vector.tensor_scalar_add(kt_sb[0][:, 0:NCH], ps_k0[:, :NCH],
                                        bqk_sb[:, 2:3])
            nc.vector.tensor_scalar_add(qt_sb[0][:, 0:NCH], ps_q0[:, :NCH],
                                        bqk_sb[:, 0:1])

            # ---- emission helpers ----
            def gen_qk_chunk(which, p, nck):
                """One Q^T (which=0) / K^T (which=1) chunk, k-inner."""
                w_f = wq_s if which == 0 else wk_s
                dst = qt_sb[p] if which == 0 else kt_sb[p]
                bcol = which * 2 + p
                ps = mm_pool.tile([128, 512], f32, tag="mm",
                                  name=f"psqk{which}{p}{nck}")
                for k in range(KT):
                    nc.tensor.matmul(
                        ps[:, :NCH],
                        lhsT=w_f(p, k),
                        rhs=xt_sb[k][:, nck * NCH:(nck + 1) * NCH],
                        start=(k == 0), stop=(k == KT - 1))
                    yield
                nc.vector.tensor_scalar_add(
                    dst[:, nck * NCH:(nck + 1) * NCH], ps[:, :NCH],
                    bqk_sb[:, bcol:bcol + 1])
                yield

            def gen_v(s0, s1):
                """V blocks s0..s1-1; bias added during the psum cast."""
                for s in range(s0, s1):
                    ps = mm_pool.tile([128, 512], f32, tag="mm", name=f"psv{s}")
                    for k in range(KT):
                        nc.tensor.matmul(
                            ps[:, :J],
                            lhsT=xt_sb[k][:, s * 128:(s + 1) * 128],
                            rhs=wv_sb[k],
                            start=(k == 0), stop=(k == KT - 1))
                        yield
                    nc.vector.tensor_add(
                        v_sb[s][:, :, 0:DH],
                        ps[:, :J].rearrange("p (h d) -> p h d", h=HL),
                        bvr_sb.rearrange("p (h d) -> p h d", h=HL))
                    yield

            pt_tiles = {}

            class Gen:
                def __init__(self, it):
                    self.it = it
                    self.done = False

                def step(self):
                    if self.done:
                        return False
                    try:
                        next(self.it)
                        return True
                    except StopIteration:
                        self.done = True
                        return False

            pending = []

            def pump(n):
                while n > 0 and pending:
                    if pending[0].step():
                        n -= 1
                    else:
                        pending.pop(0)

            def drain(g):
                while g.step():
                    pass

            def emit_sT(p, c, av, base_pump=4, av_pump=2):
                """Scores^T + exp for head-pair p, query chunk c."""
                pt = pt_pool.tile([128, KB, 2, QC], bf16, tag="pt",
                                  name=f"pt{p}{c}")
                pt_tiles[(p, c)] = pt
                for kb in range(KB):
                    ps = psS_pool.tile([128, 2, QC], f32, tag="psS",
                                       name=f"psS{p}{c}{kb}")
                    for h01 in range(2):
                        nc.tensor.matmul(
                            ps[:, h01, :],
                            lhsT=kt_sb[p][h01 * 64:(h01 + 1) * 64,
                                          kb * 128:(kb + 1) * 128],
                            rhs=qt_sb[p][h01 * 64:(h01 + 1) * 64,
                                         c * QC:(c + 1) * QC],
                            start=True, stop=True,
                            tile_position=(h01 * 64, 0))
                    nc.scalar.activation(
                        out=pt[:, kb, :, :], in_=ps,
                        func=AF.Exp, scale=0.125)
                    if av is not None and not av.done:
                        av.step()
                        av.step()
                        pump(av_pump)
                    else:
                        pump(base_pump)

            def gen_av(p, c):
                """attn @ [V|1], reciprocal, normalize, build O^T pair tile."""
                pt = pt_tiles.pop((p, c))
                for h01 in range(2):
                    h = p * 2 + h01
                    pso = psO_pool.tile([128, QC], f32, tag="psO",
                                        name=f"psO{p}{c}{h01}")
                    for kb in range(KB):
                        nc.tensor.matmul(
                            pso,
                            lhsT=v_sb[kb][:, h, :],
                            rhs=pt[:, kb, h01, :],
                            start=(kb == 0), stop=(kb == KB - 1))
                        yield
                    osb = sm_pool.tile([DH + 1, QC], f32, tag="osb",
                                       name=f"osb{p}{c}{h01}")
                    nc.vector.tensor_copy(osb, pso[0:DH + 1, :])
                    yield
                    ds = dram_pool.tile([1, QC], f32, tag="ds",
                                        name=f"ds{p}{c}{h01}")
                    nc.sync.dma_start(out=ds, in_=osb[DH:DH + 1, :])
                    dsap = ds[0:1, :]
                    rbs = sm_pool.tile([64, QC], f32, tag="rbs",
                                       name=f"rbs{p}{c}{h01}")
                    nc.sync.dma_start(
                        out=rbs,
                        in_=bass.AP(tensor=dsap.tensor, offset=dsap.offset,
                                    ap=[[0, 64], [1, QC]]))
                    rb = sm_pool.tile([64, QC], f32, tag="rb",
                                      name=f"rb{p}{c}{h01}")
                    nc.vector.reciprocal_approx_fast(out=rb, in_=rbs)
                    if h01 == 0:
                        nc.vector.tensor_mul(
                            ot_sb[p][0:64, c * QC:(c + 1) * QC],
                            osb[0:DH, :], rb)
                    else:
                        tmp = sm_pool.tile([64, QC], bf16, tag="ottmp",
                                           name=f"ottmp{p}{c}")
                        nc.vector.tensor_mul(tmp, osb[0:DH, :], rb)
                        nc.sync.dma_start(
                            out=ot_sb[p][64:128, c * QC:(c + 1) * QC],
                            in_=tmp)

            def gen_av_kb(p, c, holder):
                """attn@V accumulation only (interleaved h01), for the LAST
                unit; the normalize tail is emitted explicitly by the
                scheduler so y-filler can interleave with its latency."""
                pt = pt_tiles.pop((p, c))
                psos = [psO_pool.tile([128, QC], f32, tag="psO",
                                      name=f"psOil{h01}") for h01 in range(2)]
                holder.extend(psos)
                for kb in range(KB):
                    for h01 in range(2):
                        nc.tensor.matmul(
                            psos[h01],
                            lhsT=v_sb[kb][:, p * 2 + h01, :],
                            rhs=pt[:, kb, h01, :],
                            start=(kb == 0), stop=(kb == KB - 1))
                    yield

            def emit_sT_last(p, c, av, selfav):
                """Last unit: previous attn@V first, then own interleaved
                attn@V one slot behind the exp stream."""
                pt = pt_pool.tile([128, KB, 2, QC], bf16, tag="pt",
                                  name=f"pt{p}{c}")
                pt_tiles[(p, c)] = pt
                for kb in range(KB):
                    ps = psS_pool.tile([128, 2, QC], f32, tag="psS",
                                       name=f"psS{p}{c}{kb}")
                    for h01 in range(2):
                        nc.tensor.matmul(
                            ps[:, h01, :],
                            lhsT=kt_sb[p][h01 * 64:(h01 + 1) * 64,
                                          kb * 128:(kb + 1) * 128],
                            rhs=qt_sb[p][h01 * 64:(h01 + 1) * 64,
                                         c * QC:(c + 1) * QC],
                            start=True, stop=True,
                            tile_position=(h01 * 64, 0))
                    nc.scalar.activation(
                        out=pt[:, kb, :, :], in_=ps,
                        func=AF.Exp, scale=0.125)
                    if av is not None and not av.done:
                        av.step()
                        av.step()
                        av.step()
                        av.step()
                    elif kb >= 10:
                        for _ in range(3):
                            if not selfav.step():
                                pump(1)
                    else:
                        pump(3)

            def gen_y(c, scalar_copy=False, qbs=None, alt_pool=False,
                      p_order=None):
                """Output-projection partials for the query blocks of chunk c."""
                p1 = (psO_pool, "psO") if alt_pool else (mm_pool, "mm")
                porder = list(range(PAIRS)) if p_order is None else list(p_order)
                for qb in (range(c * (QC // 128), (c + 1) * (QC // 128))
                           if qbs is None else qbs):
                    yb = yb_pool.tile([128, D], bf16, tag="yb", name=f"yb{qb}")
                    pss = [mm_pool.tile([128, 512], f32, tag="mm",
                                        name=f"psy{qb}0"),
                           p1[0].tile([128, 512], f32, tag=p1[1],
                                      name=f"psy{qb}1")]
                    for pi, p in enumerate(porder):
                        for n in range(2):
                            nc.tensor.matmul(
                                pss[n],
                                lhsT=ot_sb[p][:, qb * 128:(qb + 1) * 128],
                                rhs=wo_sb[p][:, n * 512:(n + 1) * 512],
                                start=(pi == 0), stop=(pi == PAIRS - 1))
                            yield
                    nc.vector.tensor_copy(yb[:, 0:512], pss[0])
                    if scalar_copy:
                        nc.scalar.copy(yb[:, 512:1024], pss[1])
                    else:
                        nc.vector.tensor_copy(yb[:, 512:1024], pss[1])
                    nc.sync.dma_start(out=y_d[qb * 128:(qb + 1) * 128, :], in_=yb)

            # ---- emission schedule ----
            order = [(0, 0), (0, 1), (0, 2), (1, 0), (1, 1), (1, 2),
                     (0, 3), (1, 3)]
            # Deadline-ordered feed per unit.  xt quarter q gates kt/qt
            # chunk q and V blocks 4q..4q+3, so the interleave below only
            # emits work whose data will have landed.
            feed = {
                # unit 1 (0,0): kt0 c1-c3 (slot deadlines 4/8/12), V
                # (attn@V of u1 consumes v_sb from u2 slot 0), qt0 c1 (u2).
                0: [Gen(gen_qk_chunk(1, 0, 1)), Gen(gen_v(0, 2)),
                    Gen(gen_qk_chunk(1, 0, 2)), Gen(gen_v(2, 4)),
                    Gen(gen_qk_chunk(1, 0, 3)), Gen(gen_v(4, 8)),
                    Gen(gen_qk_chunk(0, 0, 1)), Gen(gen_v(8, 15))],
                # unit 2 (0,1): V tail (attn@V h0 reads v_sb[15] at ~slot
                # 7.5; 9 quanta at 2/slot finish by slot 4.5), then qt0 c2
                # (u3 slot 0), qt1 c0 + kt1 c0 (u4).
                1: [Gen(gen_v(15, 16)), Gen(gen_qk_chunk(0, 0, 2)),
                    Gen(gen_qk_chunk(0, 1, 0)), Gen(gen_qk_chunk(1, 1, 0))],
                # unit 3 (0,2): kt1 rest (u4 slots 4/8/12).
                2: [Gen(gen_qk_chunk(1, 1, 1)), Gen(gen_qk_chunk(1, 1, 2)),
                    Gen(gen_qk_chunk(1, 1, 3))],
                # unit 4 (1,0): qt1 c1 (u5).
                3: [Gen(gen_qk_chunk(0, 1, 1))],
                # unit 5 (1,1): qt1 c2 (u6).
                4: [Gen(gen_qk_chunk(0, 1, 2))],
                # unit 6 (1,2): qt0 c3 (u7); y(c0) joins after u5's drain.
                5: [Gen(gen_qk_chunk(0, 0, 3))],
                # unit 7 (0,3): qt1 c3 (u8); y(c1) joins.
                6: [Gen(gen_qk_chunk(0, 1, 3))],
                # unit 8 (1,3): y(c2) first half joins; second half reserved.
                7: [],
            }

            av = None
            prev = None
            y_after = {(1, 0): 0, (1, 1): 1, (1, 2): 2}
            reserved = []
            for ui, (p, c) in enumerate(order):
                pending.extend(feed[ui])
                if ui == len(order) - 1:
                    psos = []
                    selfav = Gen(gen_av_kb(p, c, psos))
                    emit_sT_last(p, c, av, selfav)
                    if av is not None:
                        drain(av)
                    drain(selfav)

                    # Explicit tail: normalize each half with a
                    # PE-broadcast reciprocal (ones[1,64] K=1 matmul
                    # replaces the slow DRAM-replicate bounce) and drain
                    # reserved y(c2) blocks between the DVE steps so the
                    # PE never idles long enough to drop its p-state.
                    def norm_a(h01):
                        osb = sm_pool.tile([DH + 1, QC], f32, tag="osb",
                                           name=f"osbT{h01}")
                        nc.vector.tensor_copy(osb, psos[h01][0:DH + 1, :])
                        ds = dram_pool.tile([1, QC], f32, tag="ds",
                                            name=f"dsT{h01}")
                        nc.sync.dma_start(out=ds, in_=osb[DH:DH + 1, :])
                        dsap = ds[0:1, :]
                        rbs = sm_pool.tile([64, QC], f32, tag="rbs",
                                           name=f"rbsT{h01}")
                        nc.sync.dma_start(
                            out=rbs,
                            in_=bass.AP(tensor=dsap.tensor, offset=dsap.offset,
                                        ap=[[0, 64], [1, QC]]))
                        return osb, rbs

                    def norm_b(h01, osb, rbs):
                        rbp = sm_pool.tile([64, QC], f32, tag="rb",
                                           name=f"rbT{h01}")
                        nc.vector.reciprocal_approx_fast(out=rbp, in_=rbs)
                        if h01 == 0:
                            nc.vector.tensor_mul(
                                ot_sb[p][0:64, c * QC:(c + 1) * QC],
                                osb[0:DH, :], rbp)
                        else:
                            tmp = sm_pool.tile([64, QC], bf16, tag="ottmp",
                                               name="ottmpT")
                            nc.vector.tensor_mul(tmp, osb[0:DH, :],
                                                 rbp)
                            nc.sync.dma_start(
                                out=ot_sb[p][64:128, c * QC:(c + 1) * QC],
                                in_=tmp)

                    osb1, rbs1 = norm_a(1)
                    if reserved:
                        drain(reserved[0])
                    norm_b(1, osb1, rbs1)
                    osb0, rbs0 = norm_a(0)
                    for g in reserved[1:]:
                        drain(g)
                    norm_b(0, osb0, rbs0)
                    pump(1 << 30)

                    # Final y: two passes per 2-qb group - pair 0 (whose ot
                    # was normalized during this unit's slots) streams first
                    # so the PE never stalls head-of-line on the pair-1 ot
                    # still in the normalize bounce.
                    qbs = [c * 4 + i for i in range(4)]
                    for gi, half in enumerate((qbs[0:2], qbs[2:4])):
                        tiles = {}
                        for qb in half:
                            yb = yb_pool.tile([128, D], bf16, tag="yb",
                                              name=f"ybF{qb}")
                            pss = [mm_pool.tile([128, 512], f32, tag="mm",
                                                name=f"psyF{qb}0"),
                                   psO_pool.tile([128, 512], f32, tag="psO",
                                                 name=f"psyF{qb}1")]
                            tiles[qb] = (yb, pss)
                        for qb in half:
                            for n in range(2):
                                nc.tensor.matmul(
                                    tiles[qb][1][n],
                                    lhsT=ot_sb[0][:, qb * 128:(qb + 1) * 128],
                                    rhs=wo_sb[0][:, n * 512:(n + 1) * 512],
                                    start=True, stop=False)
                        for qb in half:
                            yb, pss = tiles[qb]
                            for n in range(2):
                                nc.tensor.matmul(
                                    pss[n],
                                    lhsT=ot_sb[1][:, qb * 128:(qb + 1) * 128],
                                    rhs=wo_sb[1][:, n * 512:(n + 1) * 512],
                                    start=False, stop=True)
                            nc.vector.tensor_copy(yb[:, 0:512], pss[0])
                            nc.sync.dma_start(
                                out=y_d[qb * 128:(qb + 1) * 128, 0:512],
                                in_=yb[:, 0:512])
                            nc.scalar.copy(yb[:, 512:1024], pss[1])
                            nc.sync.dma_start(
                                out=y_d[qb * 128:(qb + 1) * 128, 512:1024],
                                in_=yb[:, 512:1024])
                else:
                    emit_sT(p, c, av, base_pump=(11 if prev is None else 4),
                            av_pump=2)
                    if av is not None:
                        drain(av)
                    if prev in y_after:
                        yc = y_after[prev]
                        if yc == 2:
                            pending.append(Gen(gen_y(2, qbs=[8])))
                            for qb in (9, 10, 11):
                                reserved.append(Gen(gen_y(2, qbs=[qb],
                                                          scalar_copy=True,
                                                          alt_pool=True)))
                        elif yc == 1:
                            # reserve two blocks of y(c1) as extra tail
                            # filler: the final normalize chain (~6us) needs
                            # more PE coverage than y(c2) alone provides.
                            pending.append(Gen(gen_y(1, qbs=[4, 5])))
                            for qb in (6, 7):
                                reserved.append(Gen(gen_y(1, qbs=[qb],
                                                          scalar_copy=True,
                                                          alt_pool=True)))
                        else:
                            pending.append(Gen(gen_y(yc)))
                    av = Gen(gen_av(p, c))
                prev = (p, c)
            pump(1 << 30)

    nc.compile()
    return nc


def _get_module(seq=S):
    if seq not in _cache:
        _cache[seq] = _build_module(seq)
    return _cache[seq]


def _wslab(Wq_s, Wk_s, p):
    """[wq pair p | wk pair p] as [128, KT*256] (partition-major: row d%128,
    col k*256 + c) matching the SBUF tile [128, KT, 256]."""
    KT = D // 128
    cols = slice(p * 128, (p + 1) * 128)
    wq_r = Wq_s[:, cols].reshape(KT, 128, 128)
    wk_r = Wk_s[:, cols].reshape(KT, 128, 128)
    w = np.concatenate([wq_r, wk_r], axis=2)      # [KT, 128, 256]
    return np.ascontiguousarray(w.transpose(1, 0, 2).reshape(128, KT * 256))


def _make_in_maps(x, Wq, bq, Wk, bk, Wv, bv, Wo):
    import ml_dtypes
    bf16 = ml_dtypes.bfloat16
    KT = D // 128
    in_maps = []
    for c in range(NCORES):
        b, hg = divmod(c, 4)
        js = slice(hg * J, (hg + 1) * J)
        bqs = np.asarray(bq[js], np.float32)
        bks = np.asarray(bk[js], np.float32)
        bqk = np.stack([bqs[0:128], bqs[128:256],
                        bks[0:128], bks[128:256]], axis=1)
        Wq_s = np.asarray(Wq, np.float32)[:, js]
        Wk_s = np.asarray(Wk, np.float32)[:, js]
        Wv_s = np.asarray(Wv, np.float32)[:, js]
        wv_slab = np.ascontiguousarray(
            Wv_s.reshape(KT, 128, 256).transpose(1, 0, 2).reshape(128, KT * 256))
        bvr = np.broadcast_to(np.asarray(bv[js], np.float32).reshape(1, J),
                              (128, J))
        in_maps.append({
            "xT": np.ascontiguousarray(np.asarray(x[b], np.float32).T).astype(bf16),
            "w0": _wslab(Wq_s, Wk_s, 0).astype(bf16),
            "w1": _wslab(Wq_s, Wk_s, 1).astype(bf16),
            "wv": wv_slab.astype(bf16),
            "wo": np.ascontiguousarray(np.asarray(Wo, np.float32)[js, :]).astype(bf16),
            "bqk": np.ascontiguousarray(bqk.astype(np.float32)),
            "bv": np.ascontiguousarray(bvr).astype(bf16),
        })
    return in_maps


def _gather(results, bo):
    y = np.zeros((B, S, D), np.float32)
    for b in range(B):
        acc = np.zeros((S, D), np.float32)
        for hg in range(4):
            acc += np.asarray(results[b * 4 + hg]["y"], np.float32)
        y[b] = acc + np.asarray(bo, np.float32)[None, :]
    return y


def run_on_hw(inputs, trace=False, **kwargs):
    """Returns (y_full, BassKernelResults)."""
    from concourse.bass_utils import run_bass_kernel_spmd
    nc = _get_module()
    in_maps = _make_in_maps(
        inputs["x"], inputs["Wq"], inputs["bq"], inputs["Wk"], inputs["bk"],
        inputs["Wv"], inputs["bv"], inputs["Wo"])
    res = run_bass_kernel_spmd(nc, in_maps, core_ids=list(range(NCORES)),
                               trace=trace, **kwargs)
    y = _gather(res.results, inputs["bo"])
    return y, res


def kernel(x, Wq, bq, Wk, bk, Wv, bv, Wo, bo):
    y, _ = run_on_hw(dict(x=x, Wq=Wq, bq=bq, Wk=Wk, bk=bk, Wv=Wv, bv=bv,
                          Wo=Wo, bo=bo))
    return y


# revision 34
# speedup vs baseline: 1.0450x; 1.0160x over previous
# Multi-head attention (B=2, S=2048, D=1024, H=16) on 8 TRN2 NeuronCores.
#
# Sharding: core c handles batch b = c//4 and head-group hg = c%4 (4 heads,
# channel slice J = hg*256 : (hg+1)*256).  Each core computes
#   Q^T/K^T = W^T x^T (+bias), V = x W (+bias),
#   S^T_h = K_h^T^T-contraction (d on partitions)  -> exp on ScalarE,
#   O^T_h = [V | 1]^T P^T_h  (row 64 = softmax denominator),
#   y_partial = O^T^T Wo_slice    (bf16, [S, D])
# Host sums the 4 partials per batch and adds bo.
#
# Schedule (v2): the PE stream is the binding path, so everything is
# organized to keep it gapless from ~10us on:
#  - xT and Wqkv live in single SBUF tiles [128, KT, .] so one DMA
#    instruction covers a column-slice of ALL k-tiles (3D DRAM AP).  The
#    pair-0 wq/wk columns load first (0.5MB prefix), then xt streams in
#    seq-quarters; the first scores unit starts at ~14us instead of ~32us.
#  - Weight/bias/wv DMAs ride the Scalar engine's HWDGE queue so the sync
#    engine's descriptor time stays off the critical path.
#  - Unit order (0,0),(0,1),(0,2),(1,0),(1,1),(1,2),(0,3),(1,3): y(c)
#    unlocks after the second pair of chunk c, spreading output DMAs.
#  - Pump generators are drained into the exp slots in deadline order
#    (emission order == execution order per engine); unit 1 pumps hard
#    (PE-paced) to finish V + the remaining kt0 chunks before attn@V of
#    unit 1 consumes them during unit 2.
#  - Tail: the last unit interleaves its own attn@V one slot behind the
#    exp stream; a reserved half of y(c2) covers the final normalize
#    latency so the PE p-state never drops before the last y matmuls.
#
# All matmuls bf16; scores use K=64 tile packing (two heads' MMs run
# CONCURRENT in disjoint PE row groups).  Q/K biases fold into the
# PSUM->SBUF cast (per-partition tensor_scalar add); V bias folds into its
# cast.  Softmax: denominator row 64 bounces through DRAM to replicate
# across partitions, then reciprocal_approx_fast.

import numpy as np

B = 2
S = 2048
D = 1024
H = 16
DH = 64
NCORES = 8
HL = 4            # heads per core
J = HL * DH       # 256: per-core channel slice of D
PAIRS = 2         # head-pairs per core

_cache = {}


def _build_module(seq=S):
    import concourse.bass as bass
    import concourse.mybir as mybir
    import concourse.tile as tile

    from concourse import bacc

    dt = mybir.dt
    f32 = dt.float32
    bf16 = dt.bfloat16
    AF = mybir.ActivationFunctionType

    KB = seq // 128          # key blocks (partition tiles of the key dim)
    QC = min(512, seq)       # query chunk (matmul free dim)
    NQ = seq // QC           # query chunks
    NCH = min(512, seq)      # projection free-dim chunk
    NP = seq // NCH          # projection chunks
    KT = D // 128            # contraction tiles for projections (8)

    nc = bacc.Bacc("TRN2", target_bir_lowering=False, debug=False)

    xT_d = nc.dram_tensor("xT", [D, seq], bf16, kind="ExternalInput").ap()
    # Weight slabs pre-transposed on host to [128, KT*256] (partition-major)
    # so each loads with ONE contiguous 2D DMA: w0/w1 = [wq_p|wk_p] of head
    # pair p, wv = the V weights.
    w0_d = nc.dram_tensor("w0", [128, KT * 256], bf16, kind="ExternalInput").ap()
    w1_d = nc.dram_tensor("w1", [128, KT * 256], bf16, kind="ExternalInput").ap()
    wv_d = nc.dram_tensor("wv", [128, KT * 256], bf16, kind="ExternalInput").ap()
    wo_d = nc.dram_tensor("wo", [J, D], bf16, kind="ExternalInput").ap()
    bqk_d = nc.dram_tensor("bqk", [128, 4], f32, kind="ExternalInput").ap()
    # bv pre-replicated to all 128 partitions on host.
    bv_d = nc.dram_tensor("bv", [128, J], bf16, kind="ExternalInput").ap()
    y_d = nc.dram_tensor("y", [seq, D], bf16, kind="ExternalOutput").ap()

    with tile.TileContext(nc) as tc:
        import contextlib
        ctx = contextlib.ExitStack()
        with ctx:
            xt_pool = ctx.enter_context(tc.tile_pool(name="xt", bufs=1))
            w_pool = ctx.enter_context(tc.tile_pool(name="w", bufs=1))
            qk_pool = ctx.enter_context(tc.tile_pool(name="qk", bufs=1))
            v_pool = ctx.enter_context(tc.tile_pool(name="v", bufs=1))
            pt_pool = ctx.enter_context(tc.tile_pool(name="pt", bufs=2))
            ot_pool = ctx.enter_context(tc.tile_pool(name="ot", bufs=1))
            sm_pool = ctx.enter_context(tc.tile_pool(name="sm", bufs=5))
            yb_pool = ctx.enter_context(tc.tile_pool(name="yb", bufs=5))
            psS_pool = ctx.enter_context(
                tc.tile_pool(name="psS", bufs=2, space="PSUM"))
            psO_pool = ctx.enter_context(
                tc.tile_pool(name="psO", bufs=2, space="PSUM"))
            mm_pool = ctx.enter_context(
                tc.tile_pool(name="mm", bufs=2, space="PSUM"))
            dram_pool = ctx.enter_context(
                tc.tile_pool(name="dscr", bufs=4, space="DRAM"))

            # ---- persistent SBUF tensors ----
            xt_all = xt_pool.tile([128, KT, seq], bf16, tag="xt", name="xt")
            xt_sb = [xt_all[:, k, :] for k in range(KT)]
            # wp_sb[p][:, k, 0:128] = wq pair p, [:, k, 128:256] = wk pair p
            wp_sb = [w_pool.tile([128, KT, 256], bf16, tag=f"w{p}",
                                 name=f"w{p}") for p in range(PAIRS)]
            wv_all = w_pool.tile([128, KT, 256], bf16, tag="wv", name="wv")
            wv_sb = [wv_all[:, k, :] for k in range(KT)]
            wo_sb = [w_pool.tile([128, D], bf16, tag=f"wo{p}",
                                 name=f"wo{p}") for p in range(PAIRS)]
            bqk_sb = w_pool.tile([128, 4], f32, tag="bqk", name="bqk")
            bvr_sb = w_pool.tile([128, J], bf16, tag="bvr", name="bvr")
            dz_sb = w_pool.tile([128, 512], bf16, tag="dz", name="dz")

            qt_sb = [qk_pool.tile([128, seq], bf16, tag=f"qt{p}",
                                  name=f"qt{p}") for p in range(PAIRS)]
            kt_sb = [qk_pool.tile([128, seq], bf16, tag=f"kt{p}",
                                  name=f"kt{p}") for p in range(PAIRS)]
            # V padded to 128 columns per head (NumWeights==128 -> FWL).
            v_sb = [v_pool.tile([128, HL, 128], bf16, tag=f"v{s}",
                                name=f"v{s}") for s in range(KB)]
            ot_sb = [ot_pool.tile([128, seq], bf16, tag=f"ot{p}",
                                  name=f"ot{p}") for p in range(PAIRS)]

            # ---- input DMAs ----
            def xtq(q, k0, k1):
                """One DMA for xt quarter q, k-tiles k0..k1-1."""
                x0 = xT_d[0:1, 0:1]
                nc.sync.dma_start(
                    out=xt_all[:, k0:k1, q * QC:(q + 1) * QC],
                    in_=bass.AP(tensor=x0.tensor,
                                offset=x0.offset + k0 * 128 * seq + q * QC,
                                ap=[[seq, 128], [128 * seq, k1 - k0],
                                    [1, QC]]))

            # Sync queue, deadline order: pair-0 weights + xt quarter 0
            # (gate the prefix chains), bqk (prefix bias), wv + bv (V
            # blocks from ~slot 2), then the later quarters and weights.
            # Sync HWDGE queue (one queue sustains only ~100-150 GB/s
            # serially): the prefix w0 + xt stream in deadline order.  The
            # late-needed weights (w1, wo: ~1MB) ride the gpsimd SWDGE
            # queue in parallel so they don't delay the xt quarters.
            nc.sync.dma_start(out=wp_sb[0], in_=w0_d)
            for kp in range(KT // 2):
                xtq(0, 2 * kp, 2 * kp + 2)
            nc.sync.dma_start(out=bqk_sb, in_=bqk_d)
            nc.sync.dma_start(out=wv_all, in_=wv_d)
            nc.sync.dma_start(out=bvr_sb, in_=bv_d)
            for kp in range(KT // 2):
                xtq(1, 2 * kp, 2 * kp + 2)
            nc.sync.dma_start(out=wp_sb[1], in_=w1_d)
            xtq(2, 0, KT // 2)
            xtq(2, KT // 2, KT)
            for pp in range(PAIRS):
                nc.sync.dma_start(out=wo_sb[pp],
                                  in_=wo_d[pp * 128:(pp + 1) * 128, :])
            xtq(3, 0, KT // 2)
            xtq(3, KT // 2, KT)

            # PE p-state warm-up: dummy matmuls on a zeroed tile ramp the
            # tensor clock while the first DMAs land.
            nc.vector.memset(dz_sb, 0.0)
            trash = psS_pool.tile([128, 2, QC], f32, tag="psS", name="trash")
            for i in range(14):
                nc.tensor.matmul(trash[:, 0, :], lhsT=dz_sb[:, 0:128],
                                 rhs=dz_sb, start=True, stop=True)
            for s in range(KB):
                nc.vector.memset(v_sb[s][:, :, DH:DH + 1], 1.0)
                nc.vector.memset(v_sb[s][:, :, DH + 1:], 0.0)
            # Warm the exp table set during the DMA ramp.
            warm = w_pool.tile([1, 8], f32, tag="warm", name="warm")
            nc.vector.memset(warm, 0.0)
            nc.scalar.activation(out=warm, in_=warm, func=AF.Exp)

            def wq_s(p, k):
                return wp_sb[p][:, k, 0:128]

            def wk_s(p, k):
                return wp_sb[p][:, k, 128:256]

            # ---- prefix: kt0 chunk 0 + qt0 chunk 0, k-outer (DMA-paced) ----
            ps_k0 = mm_pool.tile([128, 512], f32, tag="mm", name="pfx_k")
            ps_q0 = mm_pool.tile([128, 512], f32, tag="mm", name="pfx_q")
            for k in range(KT):
                nc.tensor.matmul(ps_k0[:, :NCH], lhsT=wk_s(0, k),
                                 rhs=xt_sb[k][:, 0:NCH],
                                 start=(k == 0), stop=(k == KT - 1))
                nc.tensor.matmul(ps_q0[:, :NCH], lhsT=wq_s(0, k),
                                 rhs=xt_sb[k][:, 0:NCH],
                                 start=(k == 0), stop=(k == KT - 1))
            nc.vector.tensor_scalar_add(kt_sb[0][:, 0:NCH], ps_k0[:, :NCH],
                                        bqk_sb[:, 2:3])
            nc.vector.tensor_scalar_add(qt_sb[0][:, 0:NCH], ps_q0[:, :NCH],
                                        bqk_sb[:, 0:1])

            # ---- emission helpers ----
            def gen_qk_chunk(which, p, nck):
                """One Q^T (which=0) / K^T (which=1) chunk, k-inner."""
                w_f = wq_s if which == 0 else wk_s
                dst = qt_sb[p] if which == 0 else kt_sb[p]
                bcol = which * 2 + p
                ps = mm_pool.tile([128, 512], f32, tag="mm",
                                  name=f"psqk{which}{p}{nck}")
                for k in range(KT):
                    nc.tensor.matmul(
                        ps[:, :NCH],
                        lhsT=w_f(p, k),
                        rhs=xt_sb[k][:, nck * NCH:(nck + 1) * NCH],
                        start=(k == 0), stop=(k == KT - 1))
                    yield
                nc.vector.tensor_scalar_add(
                    dst[:, nck * NCH:(nck + 1) * NCH], ps[:, :NCH],
                    bqk_sb[:, bcol:bcol + 1])
                yield

            def gen_v(s0, s1):
                """V blocks s0..s1-1; bias added during the psum cast."""
                for s in range(s0, s1):
                    ps = mm_pool.tile([128, 512], f32, tag="mm", name=f"psv{s}")
                    for k in range(KT):
                        nc.tensor.matmul(
                            ps[:, :J],
                            lhsT=xt_sb[k][:, s * 128:(s + 1) * 128],
                            rhs=wv_sb[k],
                            start=(k == 0), stop=(k == KT - 1))
                        yield
                    nc.vector.tensor_add(
                        v_sb[s][:, :, 0:DH],
                        ps[:, :J].rearrange("p (h d) -> p h d", h=HL),
                        bvr_sb.rearrange("p (h d) -> p h d", h=HL))
                    yield

            pt_tiles = {}

            class Gen:
                def __init__(self, it):
                    self.it = it
                    self.done = False

                def step(self):
                    if self.done:
                        return False
                    try:
                        next(self.it)
                        return True
                    except StopIteration:
                        self.done = True
                        return False

            pending = []

            def pump(n):
                while n > 0 and pending:
                    if pending[0].step():
                        n -= 1
                    else:
                        pending.pop(0)

            def drain(g):
                while g.step():
                    pass

            def emit_sT(p, c, av, base_pump=4, av_pump=2):
                """Scores^T + exp for head-pair p, query chunk c."""
                pt = pt_pool.tile([128, KB, 2, QC], bf16, tag="pt",
                                  name=f"pt{p}{c}")
                pt_tiles[(p, c)] = pt
                for kb in range(KB):
                    ps = psS_pool.tile([128, 2, QC], f32, tag="psS",
                                       name=f"psS{p}{c}{kb}")
                    for h01 in range(2):
                        nc.tensor.matmul(
                            ps[:, h01, :],
                            lhsT=kt_sb[p][h01 * 64:(h01 + 1) * 64,
                                          kb * 128:(kb + 1) * 128],
                            rhs=qt_sb[p][h01 * 64:(h01 + 1) * 64,
                                         c * QC:(c + 1) * QC],
                            start=True, stop=True,
                            tile_position=(h01 * 64, 0))
                    nc.scalar.activation(
                        out=pt[:, kb, :, :], in_=ps,
                        func=AF.Exp, scale=0.125)
                    if av is not None and not av.done:
                        av.step()
                        av.step()
                        pump(av_pump)
                    else:
                        pump(base_pump)

            def gen_av(p, c):
                """attn @ [V|1], reciprocal, normalize, build O^T pair tile."""
                pt = pt_tiles.pop((p, c))
                for h01 in range(2):
                    h = p * 2 + h01
                    pso = psO_pool.tile([128, QC], f32, tag="psO",
                                        name=f"psO{p}{c}{h01}")
                    for kb in range(KB):
                        nc.tensor.matmul(
                            pso,
                            lhsT=v_sb[kb][:, h, :],
                            rhs=pt[:, kb, h01, :],
                            start=(kb == 0), stop=(kb == KB - 1))
                        yield
                    osb = sm_pool.tile([DH + 1, QC], f32, tag="osb",
                                       name=f"osb{p}{c}{h01}")
                    nc.vector.tensor_copy(osb, pso[0:DH + 1, :])
                    yield
                    ds = dram_pool.tile([1, QC], f32, tag="ds",
                                        name=f"ds{p}{c}{h01}")
                    nc.sync.dma_start(out=ds, in_=osb[DH:DH + 1, :])
                    dsap = ds[0:1, :]
                    rbs = sm_pool.tile([64, QC], f32, tag="rbs",
                                       name=f"rbs{p}{c}{h01}")
                    nc.sync.dma_start(
                        out=rbs,
                        in_=bass.AP(tensor=dsap.tensor, offset=dsap.offset,
                                    ap=[[0, 64], [1, QC]]))
                    rb = sm_pool.tile([64, QC], f32, tag="rb",
                                      name=f"rb{p}{c}{h01}")
                    nc.vector.reciprocal_approx_fast(out=rb, in_=rbs)
                    if h01 == 0:
                        nc.vector.tensor_mul(
                            ot_sb[p][0:64, c * QC:(c + 1) * QC],
                            osb[0:DH, :], rb)
                    else:
                        tmp = sm_pool.tile([64, QC], bf16, tag="ottmp",
                                           name=f"ottmp{p}{c}")
                        nc.vector.tensor_mul(tmp, osb[0:DH, :], rb)
                        nc.sync.dma_start(
                            out=ot_sb[p][64:128, c * QC:(c + 1) * QC],
                            in_=tmp)

            def gen_av_kb(p, c, holder):
                """attn@V accumulation only (interleaved h01), for the LAST
                unit; the normalize tail is emitted explicitly by the
                scheduler so y-filler can interleave with its latency."""
                pt = pt_tiles.pop((p, c))
                psos = [psO_pool.tile([128, QC], f32, tag="psO",
                                      name=f"psOil{h01}") for h01 in range(2)]
                holder.extend(psos)
                for kb in range(KB):
                    for h01 in range(2):
                        nc.tensor.matmul(
                            psos[h01],
                            lhsT=v_sb[kb][:, p * 2 + h01, :],
                            rhs=pt[:, kb, h01, :],
                            start=(kb == 0), stop=(kb == KB - 1))
                    yield

            def emit_sT_last(p, c, av, selfav):
                """Last unit: previous attn@V first, then own interleaved
                attn@V one slot behind the exp stream."""
                pt = pt_pool.tile([128, KB, 2, QC], bf16, tag="pt",
                                  name=f"pt{p}{c}")
                pt_tiles[(p, c)] = pt
                for kb in range(KB):
                    ps = psS_pool.tile([128, 2, QC], f32, tag="psS",
                                       name=f"psS{p}{c}{kb}")
                    for h01 in range(2):
                        nc.tensor.matmul(
                            ps[:, h01, :],
                            lhsT=kt_sb[p][h01 * 64:(h01 + 1) * 64,
                                          kb * 128:(kb + 1) * 128],
                            rhs=qt_sb[p][h01 * 64:(h01 + 1) * 64,
                                         c * QC:(c + 1) * QC],
                            start=True, stop=True,
                            tile_position=(h01 * 64, 0))
                    nc.scalar.activation(
                        out=pt[:, kb, :, :], in_=ps,
                        func=AF.Exp, scale=0.125)
                    if av is not None and not av.done:
                        av.step()
                        av.step()
                        av.step()
                        av.step()
                    elif kb >= 10:
                        for _ in range(3):
                            if not selfav.step():
                                pump(1)
                    else:
                        pump(3)

            def gen_y(c, scalar_copy=False, qbs=None, alt_pool=False,
                      p_order=None):
                """Output-projection partials for the query blocks of chunk c."""
                p1 = (psO_pool, "psO") if alt_pool else (mm_pool, "mm")
                porder = list(range(PAIRS)) if p_order is None else list(p_order)
                for qb in (range(c * (QC // 128), (c + 1) * (QC // 128))
                           if qbs is None else qbs):
                    yb = yb_pool.tile([128, D], bf16, tag="yb", name=f"yb{qb}")
                    pss = [mm_pool.tile([128, 512], f32, tag="mm",
                                        name=f"psy{qb}0"),
                           p1[0].tile([128, 512], f32, tag=p1[1],
                                      name=f"psy{qb}1")]
                    for pi, p in enumerate(porder):
                        for n in range(2):
                            nc.tensor.matmul(
                                pss[n],
                                lhsT=ot_sb[p][:, qb * 128:(qb + 1) * 128],
                                rhs=wo_sb[p][:, n * 512:(n + 1) * 512],
                                start=(pi == 0), stop=(pi == PAIRS - 1))
                            yield
                    nc.vector.tensor_copy(yb[:, 0:512], pss[0])
                    if scalar_copy:
                        nc.scalar.copy(yb[:, 512:1024], pss[1])
                    else:
                        nc.vector.tensor_copy(yb[:, 512:1024], pss[1])
                    nc.sync.dma_start(out=y_d[qb * 128:(qb + 1) * 128, :], in_=yb)

            # ---- emission schedule ----
            order = [(0, 0), (0, 1), (0, 2), (1, 0), (1, 1), (1, 2),
                     (0, 3), (1, 3)]
            # Deadline-ordered feed per unit.  xt quarter q gates kt/qt
            # chunk q and V blocks 4q..4q+3, so the interleave below only
            # emits work whose data will have landed.
            feed = {
                # unit 1 (0,0): kt0 c1-c3 (slot deadlines 4/8/12), V
                # (attn@V of u1 consumes v_sb from u2 slot 0), qt0 c1 (u2).
                0: [Gen(gen_qk_chunk(1, 0, 1)), Gen(gen_v(0, 2)),
                    Gen(gen_qk_chunk(1, 0, 2)), Gen(gen_v(2, 4)),
                    Gen(gen_qk_chunk(1, 0, 3)), Gen(gen_v(4, 8)),
                    Gen(gen_qk_chunk(0, 0, 1)), Gen(gen_v(8, 15))],
                # unit 2 (0,1): V tail (attn@V h0 reads v_sb[15] at ~slot
                # 7.5; 9 quanta at 2/slot finish by slot 4.5), then qt0 c2
                # (u3 slot 0), qt1 c0 + kt1 c0 (u4).
                1: [Gen(gen_v(15, 16)), Gen(gen_qk_chunk(0, 0, 2)),
                    Gen(gen_qk_chunk(0, 1, 0)), Gen(gen_qk_chunk(1, 1, 0))],
                # unit 3 (0,2): kt1 rest (u4 slots 4/8/12).
                2: [Gen(gen_qk_chunk(1, 1, 1)), Gen(gen_qk_chunk(1, 1, 2)),
                    Gen(gen_qk_chunk(1, 1, 3))],
                # unit 4 (1,0): qt1 c1 (u5).
                3: [Gen(gen_qk_chunk(0, 1, 1))],
                # unit 5 (1,1): qt1 c2 (u6).
                4: [Gen(gen_qk_chunk(0, 1, 2))],
                # unit 6 (1,2): qt0 c3 (u7); y(c0) joins after u5's drain.
                5: [Gen(gen_qk_chunk(0, 0, 3))],
                # unit 7 (0,3): qt1 c3 (u8); y(c1) joins.
                6: [Gen(gen_qk_chunk(0, 1, 3))],
                # unit 8 (1,3): y(c2) first half joins; second half reserved.
                7: [],
            }

            av = None
            prev = None
            y_after = {(1, 0): 0, (1, 1): 1, (1, 2): 2}
            reserved = []
            for ui, (p, c) in enumerate(order):
                pending.extend(feed[ui])
                if ui == len(order) - 1:
                    psos = []
                    selfav = Gen(gen_av_kb(p, c, psos))
                    emit_sT_last(p, c, av, selfav)
                    if av is not None:
                        drain(av)
                    drain(selfav)

                    # Explicit tail: normalize each half with a
                    # PE-broadcast reciprocal (ones[1,64] K=1 matmul
                    # replaces the slow DRAM-replicate bounce) and drain
                    # reserved y(c2) blocks between the DVE steps so the
                    # PE never idles long enough to drop its p-state.
                    def norm_a(h01):
                        osb = sm_pool.tile([DH + 1, QC], f32, tag="osb",
                                           name=f"osbT{h01}")
                        nc.vector.tensor_copy(osb, psos[h01][0:DH + 1, :])
                        ds = dram_pool.tile([1, QC], f32, tag="ds",
                                            name=f"dsT{h01}")
                        nc.sync.dma_start(out=ds, in_=osb[DH:DH + 1, :])
                        dsap = ds[0:1, :]
                        rbs = sm_pool.tile([64, QC], f32, tag="rbs",
                                           name=f"rbsT{h01}")
                        nc.sync.dma_start(
                            out=rbs,
                            in_=bass.AP(tensor=dsap.tensor, offset=dsap.offset,
                                        ap=[[0, 64], [1, QC]]))
                        return osb, rbs

                    def norm_b(h01, osb, rbs):
                        rbp = sm_pool.tile([64, QC], f32, tag="rb",
                                           name=f"rbT{h01}")
                        nc.vector.reciprocal_approx_fast(out=rbp, in_=rbs)
                        if h01 == 0:
                            nc.vector.tensor_mul(
                                ot_sb[p][0:64, c * QC:(c + 1) * QC],
                                osb[0:DH, :], rbp)
                        else:
                            tmp = sm_pool.tile([64, QC], bf16, tag="ottmp",
                                               name="ottmpT")
                            nc.vector.tensor_mul(tmp, osb[0:DH, :],
                                                 rbp)
                            nc.sync.dma_start(
                                out=ot_sb[p][64:128, c * QC:(c + 1) * QC],
                                in_=tmp)

                    osb1, rbs1 = norm_a(1)
                    if reserved:
                        drain(reserved[0])
                    norm_b(1, osb1, rbs1)
                    osb0, rbs0 = norm_a(0)
                    for g in reserved[1:]:
                        drain(g)
                    norm_b(0, osb0, rbs0)
                    pump(1 << 30)

                    # Final y: two passes per 2-qb group - pair 0 (whose ot
                    # was normalized during this unit's slots) streams first
                    # so the PE never stalls head-of-line on the pair-1 ot
                    # still in the normalize bounce.
                    qbs = [c * 4 + i for i in range(4)]
                    for gi, half in enumerate((qbs[0:2], qbs[2:4])):
                        tiles = {}
                        for qb in half:
                            yb = yb_pool.tile([128, D], bf16, tag="yb",
                                              name=f"ybF{qb}")
                            pss = [mm_pool.tile([128, 512], f32, tag="mm",
                                                name=f"psyF{qb}0"),
                                   psO_pool.tile([128, 512], f32, tag="psO",
                                                 name=f"psyF{qb}1")]
                            tiles[qb] = (yb, pss)
                        for qb in half:
                            for n in range(2):
                                nc.tensor.matmul(
                                    tiles[qb][1][n],
                                    lhsT=ot_sb[0][:, qb * 128:(qb + 1) * 128],
                                    rhs=wo_sb[0][:, n * 512:(n + 1) * 512],
                                    start=True, stop=False)
                        for qb in half:
                            yb, pss = tiles[qb]
                            for n in range(2):
                                nc.tensor.matmul(
                                    pss[n],
                                    lhsT=ot_sb[1][:, qb * 128:(qb + 1) * 128],
                                    rhs=wo_sb[1][:, n * 512:(n + 1) * 512],
                                    start=False, stop=True)
                            nc.vector.tensor_copy(yb[:, 0:512], pss[0])
                            nc.sync.dma_start(
                                out=y_d[qb * 128:(qb + 1) * 128, 0:512],
                                in_=yb[:, 0:512])
                            nc.scalar.copy(yb[:, 512:1024], pss[1])
                            nc.sync.dma_start(
                                out=y_d[qb * 128:(qb + 1) * 128, 512:1024],
                                in_=yb[:, 512:1024])
                else:
                    emit_sT(p, c, av, base_pump=(11 if prev is None else 4),
                            av_pump=2)
                    if av is not None:
                        drain(av)
                    if prev in y_after:
                        yc = y_after[prev]
                        if yc == 2:
                            pending.append(Gen(gen_y(2, qbs=[8])))
                            for qb in (9, 10, 11):
                                reserved.append(Gen(gen_y(2, qbs=[qb],
                                                          scalar_copy=True,
                                                          alt_pool=True)))
                        elif yc == 1:
                            # reserve two blocks of y(c1) as extra tail
                            # filler: the final normalize chain (~6us) needs
                            # more PE coverage than y(c2) alone provides.
                            pending.append(Gen(gen_y(1, qbs=[4, 5])))
                            for qb in (6, 7):
                                reserved.append(Gen(gen_y(1, qbs=[qb],
                                                          scalar_copy=True,
                                                          alt_pool=True)))
                        else:
                            # y(c0): same split - blocks 2,3 join the tail
                            # reserve pool.
                            pending.append(Gen(gen_y(0, qbs=[0, 1])))
                            for qb in (2, 3):
                                reserved.append(Gen(gen_y(0, qbs=[qb],
                                                          scalar_copy=True,
                                                          alt_pool=True)))
                    av = Gen(gen_av(p, c))
                prev = (p, c)
            pump(1 << 30)

    nc.compile()
    return nc


def _get_module(seq=S):
    if seq not in _cache:
        _cache[seq] = _build_module(seq)
    return _cache[seq]


def _wslab(Wq_s, Wk_s, p):
    """[wq pair p | wk pair p] as [128, KT*256] (partition-major: row d%128,
    col k*256 + c) matching the SBUF tile [128, KT, 256]."""
    KT = D // 128
    cols = slice(p * 128, (p + 1) * 128)
    wq_r = Wq_s[:, cols].reshape(KT, 128, 128)
    wk_r = Wk_s[:, cols].reshape(KT, 128, 128)
    w = np.concatenate([wq_r, wk_r], axis=2)      # [KT, 128, 256]
    return np.ascontiguousarray(w.transpose(1, 0, 2).reshape(128, KT * 256))


def _make_in_maps(x, Wq, bq, Wk, bk, Wv, bv, Wo):
    import ml_dtypes
    bf16 = ml_dtypes.bfloat16
    KT = D // 128
    in_maps = []
    for c in range(NCORES):
        b, hg = divmod(c, 4)
        js = slice(hg * J, (hg + 1) * J)
        bqs = np.asarray(bq[js], np.float32)
        bks = np.asarray(bk[js], np.float32)
        bqk = np.stack([bqs[0:128], bqs[128:256],
                        bks[0:128], bks[128:256]], axis=1)
        Wq_s = np.asarray(Wq, np.float32)[:, js]
        Wk_s = np.asarray(Wk, np.float32)[:, js]
        Wv_s = np.asarray(Wv, np.float32)[:, js]
        wv_slab = np.ascontiguousarray(
            Wv_s.reshape(KT, 128, 256).transpose(1, 0, 2).reshape(128, KT * 256))
        bvr = np.broadcast_to(np.asarray(bv[js], np.float32).reshape(1, J),
                              (128, J))
        in_maps.append({
            "xT": np.ascontiguousarray(np.asarray(x[b], np.float32).T).astype(bf16),
            "w0": _wslab(Wq_s, Wk_s, 0).astype(bf16),
            "w1": _wslab(Wq_s, Wk_s, 1).astype(bf16),
            "wv": wv_slab.astype(bf16),
            "wo": np.ascontiguousarray(np.asarray(Wo, np.float32)[js, :]).astype(bf16),
            "bqk": np.ascontiguousarray(bqk.astype(np.float32)),
            "bv": np.ascontiguousarray(bvr).astype(bf16),
        })
    return in_maps


def _gather(results, bo):
    y = np.zeros((B, S, D), np.float32)
    for b in range(B):
        acc = np.zeros((S, D), np.float32)
        for hg in range(4):
            acc += np.asarray(results[b * 4 + hg]["y"], np.float32)
        y[b] = acc + np.asarray(bo, np.float32)[None, :]
    return y


def run_on_hw(inputs, trace=False, **kwargs):
    """Returns (y_full, BassKernelResults)."""
    from concourse.bass_utils import run_bass_kernel_spmd
    nc = _get_module()
    in_maps = _make_in_maps(
        inputs["x"], inputs["Wq"], inputs["bq"], inputs["Wk"], inputs["bk"],
        inputs["Wv"], inputs["bv"], inputs["Wo"])
    res = run_bass_kernel_spmd(nc, in_maps, core_ids=list(range(NCORES)),
                               trace=trace, **kwargs)
    y = _gather(res.results, inputs["bo"])
    return y, res


def kernel(x, Wq, bq, Wk, bk, Wv, bv, Wo, bo):
    y, _ = run_on_hw(dict(x=x, Wq=Wq, bq=bq, Wk=Wk, bk=bk, Wv=Wv, bv=bv,
                          Wo=Wo, bo=bo))
    return y
